# revision 56
# baseline (speedup 1.0000x reference)
"""Trainium2 Bass kernel for nn_AttentionStem (sparse local attention stem).

Math per output element (b, c, h, w), window kk = (di, dj) in 4x4, PAD=2:
  E[c,h,w]   = (emb_a[c,w] + emb_b[c,h]) * emb_mix[c,h,w]
  e1_kk      = exp(v_kk^2 * E)                  (softmax-1 numerator)
  q'         = q / sum_kk(e1)                   (fold softmax-1 denom into q)
  e2_kk      = exp(q' * k_kk * e1_kk)           (softmax-2 numerator)
  out        = sum_kk(e2 * v_kk) / sum_kk(e2)

Sharding: pure data parallel, one batch element per NeuronCore (8 cores).
E is folded on the host (input prep, like the padding/weight packing).

Layout per core: SBUF partition p = 64*half + c (half = h<64 ? 0 : 1), free
dims stream (h, w).  k/v/v2 maps are built once, full-size ([128, 67*132]
fp16), from a 6-partition stacked padded input (both halves' rows), so each
1x1-conv piece is a single matmul; pieces are emitted just-in-time inside
the chunk pipeline so their PSUM->SBUF copies ride in engine slack.

The 32 row-chunks (ch=2 rows per half, n=512 positions) run through an
explicitly software-pipelined schedule — engines execute their instruction
streams in order, so stages of chunk j are emitted across cycles j..j+5
(t1/e1 @ j+1; r1/q/qp/f @ j+2; s2/e2 @ j+4; m2/r2/r3/out @ j+5) and ordered
within each cycle so every queue head waits only on >=1-cycle-old inputs.
Steady state is paced by the Activation engine running the two exps
back-to-back (~7.2us per cycle).  Engine balance (measured cost-model):
  ACT  (0.83 ns/el): e1 = exp(t1), e2 = exp(s2)            ~231 us
  DVE  (0.52 ns/el 2x fp16): t1/s2 windows, qp, out,
        reciprocals, most k/v PSUM->fp16 copies            ~223 us
  Pool (0.83 ns/el): f = e1*q' (kk-broadcast), m2 window   ~222 us
  PE   (0.42 ns/col): convs + 3 sum_kk chains of 16
        PSUM-accumulating identity matmuls (exact fp32)    ~177 us
"""
import sys, os
for _p in ("/opt/trn_rl_repo", "/root/.axon_site/_ro/trn_rl_repo"):
    if os.path.isdir(_p) and _p not in sys.path:
        sys.path.insert(0, _p)

from contextlib import ExitStack, nullcontext as _nullcm
import numpy as np

import concourse.bass as bass
import concourse.bacc as bacc
import concourse.tile as tile
from concourse import mybir
import concourse.bass_utils as bass_utils
from concourse.bass_types import AP
from concourse import masks

N_CORES = 8
B, CIN, H, W = 8, 3, 128, 128
C = 64
K, PAD, KK = 4, 2, 16
HH = H // 2                 # rows per half (64)
WP = W + 2 * PAD            # 132
MAPR = HH + K - 1           # map rows kept per half (67)
MAPC = MAPR * WP            # map cols per partition (8844)
CH = 2                      # h-rows per half per chunk
PIECE = 492                 # conv piece (<=512 psum cols), ceil(8844/18)

F32 = mybir.dt.float32
BF16 = mybir.dt.bfloat16
F16 = mybir.dt.float16
F32R = mybir.dt.float32r
MULT = mybir.AluOpType.mult
EXP = mybir.ActivationFunctionType.Exp


def _ap(base: AP, offset: int, dims):
    """Build a custom free-dim AP on a tile/dram AP, keeping its partition dim."""
    return AP(tensor=base.tensor, offset=base.offset + offset,
              ap=[list(base.ap[0])] + [list(d) for d in dims])


def build_kernel(nc, ch: int = CH, cfg=None, reps: int = 0):
    """reps>0 wraps the whole body in a hardware loop (for benchmarking)."""
    n = ch * W                      # spatial elems per partition per chunk
    nch = HH // ch                  # chunks

    xp6_d = nc.dram_tensor("xp6", [2 * CIN, MAPC], F32R, kind="ExternalInput").ap()
    w6_d = {nm: nc.dram_tensor(f"w6_{nm}", [2 * CIN, 128], F32R,
                               kind="ExternalInput").ap()
            for nm in ("q", "k", "v")}
    em_d = nc.dram_tensor("em2", [128, HH * W], F16, kind="ExternalInput").ap()
    out_d = nc.dram_tensor("out", [C, H * W], F32, kind="ExternalOutput").ap()

    with tile.TileContext(nc) as tc, ExitStack() as ctx:
        const = ctx.enter_context(tc.tile_pool(name="const", bufs=1))
        xp_p = ctx.enter_context(tc.tile_pool(name="xp", bufs=3))
        map_p = ctx.enter_context(tc.tile_pool(name="maps", bufs=1))
        kk_p = ctx.enter_context(tc.tile_pool(name="kk", bufs=6))
        sm_p = ctx.enter_context(tc.tile_pool(name="small", bufs=2))
        ps_mm = ctx.enter_context(tc.tile_pool(name="psmm", bufs=2, space="PSUM"))
        ps_pp = ctx.enter_context(tc.tile_pool(name="pspp", bufs=3, space="PSUM"))
        ps_r1 = ctx.enter_context(tc.tile_pool(name="psr1", bufs=1, space="PSUM"))
        ps_r2 = ctx.enter_context(tc.tile_pool(name="psr2", bufs=1, space="PSUM"))
        ps_r3 = ctx.enter_context(tc.tile_pool(name="psr3", bufs=1, space="PSUM"))

        # ---- constants ----
        w_t = {}
        for nm, d in w6_d.items():
            wt = const.tile([2 * CIN, 128], F32R, tag=f"w{nm}")
            nc.sync.dma_start(wt[:], d[:])
            w_t[nm] = wt
        ident = const.tile([128, 128], F32, tag="ident")
        masks.make_identity(nc, ident[:])
        id_bf = const.tile([128, 128], BF16, tag="idbf")
        nc.vector.tensor_copy(id_bf[:], ident[:])
        id_f16 = const.tile([128, 128], F16, tag="idf16")
        nc.vector.tensor_copy(id_f16[:], ident[:])

        loop_cm = tc.For_i(0, reps, 1) if reps else _nullcm()
        with loop_cm:
            # ==== k/v/v2 full maps; pieces emitted inside the early cycles ====
            k_map = map_p.tile([128, MAPC], F16, tag="kmap")
            v_map = map_p.tile([128, MAPC], F16, tag="vmap")
            v2_map = map_p.tile([128, MAPC], F16, tag="v2map")

            pieces = list(range(0, MAPC, PIECE))
            pst = {}

            def piece_mm(pi):
                pc = pieces[pi]
                pw = min(PIECE, MAPC - pc)
                xt = xp_p.tile([2 * CIN, PIECE], F32R, tag="xp")
                nc.sync.dma_start(xt[:, 0:pw], _ap(xp6_d, pc, [[1, pw]]))
                for nm in ("k", "v"):
                    pt = ps_pp.tile([128, 512], F32, tag="pp")
                    pst[nm, pi] = pt
                    nc.tensor.matmul(pt[:, 0:pw], w_t[nm][:], xt[:, 0:pw],
                                     start=True, stop=True)

            def piece_copy(pi):
                pc = pieces[pi]
                pw = min(PIECE, MAPC - pc)
                for nm, mp in (("k", k_map), ("v", v_map)):
                    pt = pst.pop((nm, pi))
                    if pi in (2, 3, 4, 5):  # ACT absorbs these pre-e1(0)
                        nc.scalar.copy(mp[:, pc:pc + pw], pt[:, 0:pw])
                    else:
                        nc.vector.tensor_copy(mp[:, pc:pc + pw], pt[:, 0:pw])
                if pi % 2 == 0:
                    nc.vector.tensor_tensor(v2_map[:, pc:pc + pw],
                                            v_map[:, pc:pc + pw],
                                            v_map[:, pc:pc + pw], MULT)
                else:
                    nc.gpsimd.tensor_tensor(v2_map[:, pc:pc + pw],
                                            v_map[:, pc:pc + pw],
                                            v_map[:, pc:pc + pw], MULT)

            # Just-in-time piece schedule: piece pi's first consumer is
            # t1(j*) at cycle j*+1; emit it one cycle earlier.
            emit_at = {}
            for pi, pc in enumerate(pieces):
                r = pc // WP
                jstar = max(0, -(-(r - 5) // 2))
                emit_at.setdefault(max(0, jstar - 1), []).append(pi)

            # ==== phase B: attention chunks, software-pipelined ====
            # Stage offsets (chunk j): DMA/A/E @ cycle j; t1/e1 @ j+1;
            # r1/q-mm/rc1/qp/f @ j+2; s2/e2 @ j+4; m2/r2/r3/rc2/out @ j+5.
            # Per-engine emission order within a cycle keeps every queue head
            # on a >=1-cycle-old dependency (in-order engines never stall on
            # same-cycle work that sits behind them).
            st = {}             # (name, j) -> tile AP

            def win(mp, j, di):
                return _ap(mp[:], (j * ch + di) * WP, [[1, K], [WP, ch], [1, W]])

            def kkslice(t, di):
                return _ap(t[:], di * K * n, [[n, K], [W, ch], [1, W]])

            def reduce(src, acc, idt):
                for kk in range(KK):
                    nc.tensor.matmul(acc[:, 0:n], idt[:],
                                     src[:, kk * n:(kk + 1) * n],
                                     start=(kk == 0), stop=(kk == KK - 1))

            for cyc in range(nch + 6):
                jA = cyc            # DMA, A, E
                jB = cyc - 1        # t1, e1
                jC = cyc - 2        # r1, q-mm, rc1, qp, f
                jD = cyc - 4        # s2, e2
                jE = cyc - 5        # m2, r2, r3, rc2, out

                # --- SP/PE: JIT map-piece DMA + matmuls first ---
                for pi in emit_at.get(cyc, []):
                    piece_mm(pi)
                # --- Pool: m2(jE) (last chunk split with DVE to cut drain) ---
                if 0 <= jE < nch:
                    e2 = st["e2", jE]
                    m2 = kk_p.tile([128, KK * n], F16, tag="m2", bufs=2)
                    st["m2", jE] = m2
                    last = jE == nch - 1
                    for di in range(K):
                        eng = nc.vector if (last and di < 2) else nc.gpsimd
                        eng.tensor_tensor(
                            kkslice(m2, di), kkslice(e2, di),
                            win(v_map, jE, di), MULT)
                # --- DVE: s2(jD) first (f is 2 cycles old) ---
                if 0 <= jD < nch:
                    f_t = st.pop(("f", jD))
                    s2 = kk_p.tile([128, KK * n], F16, tag="s2", bufs=2)
                    st["s2", jD] = s2
                    for di in range(K):
                        nc.vector.tensor_tensor(
                            kkslice(s2, di), kkslice(f_t, di),
                            win(k_map, jD, di), MULT)
                # --- PE: r1(jC) first, then q-mm(jC), r2/r3(jE) ---
                if 0 <= jC < nch:
                    r1 = ps_r1.tile([128, 512], F32, tag="r1")
                    st["r1", jC] = r1
                    reduce(st["e1", jC], r1, id_bf)
                    q_ps = ps_mm.tile([128, 512], F32, tag="mm")
                    st["q", jC] = q_ps
                    nc.tensor.matmul(q_ps[:, 0:n], w_t["q"][:],
                                     st.pop(("xq", jC))[:],
                                     start=True, stop=True)
                # --- ACT: e2(jD) then e1(jB) ---
                if 0 <= jD < nch:
                    e2 = kk_p.tile([128, KK * n], F16, tag="e2", bufs=3)
                    st["e2", jD] = e2
                    nc.scalar.activation(e2[:], st.pop(("s2", jD))[:], EXP)
                # --- DVE: t1(jB) ---
                if 0 <= jB < nch:
                    E_t = st.pop(("E", jB))
                    t1 = kk_p.tile([128, KK * n], F16, tag="t1", bufs=2)
                    st["t1", jB] = t1
                    for di in range(K):
                        nc.vector.tensor_tensor(
                            kkslice(t1, di), win(v2_map, jB, di),
                            _ap(E_t[:], 0, [[0, K], [W, ch], [1, W]]), MULT)
                if 0 <= jB < nch:
                    e1 = kk_p.tile([128, KK * n], BF16, tag="e1", bufs=3)
                    st["e1", jB] = e1
                    t1 = st.pop(("t1", jB))
                    if jB == 0:     # split so the first exp starts earlier
                        h = KK * n // 2
                        nc.scalar.activation(e1[:, 0:h], t1[:, 0:h], EXP)
                        nc.scalar.activation(e1[:, h:], t1[:, h:], EXP)
                    else:
                        nc.scalar.activation(e1[:], t1[:], EXP)
                # --- PE: r2/r3(jE) ---
                if 0 <= jE < nch:
                    r2 = ps_r2.tile([128, 512], F32, tag="r2")
                    st["r2", jE] = r2
                    reduce(st.pop(("e2", jE)), r2, id_f16)
                    r3 = ps_r3.tile([128, 512], F32, tag="r3")
                    st["r3", jE] = r3
                    reduce(st.pop(("m2", jE)), r3, id_f16)
                # --- DVE: qp(jC) = q * 1/r1, then smalls(jA), then out(jE) ---
                if 0 <= jC < nch:
                    rc1 = sm_p.tile([128, n], F32, tag="rc1")
                    nc.vector.reciprocal_approx_fast(
                        rc1[:], st.pop(("r1", jC))[:, 0:n])
                    qp = sm_p.tile([128, n], BF16, tag="qp")
                    st["qp", jC] = qp
                    nc.vector.tensor_tensor(qp[:], st.pop(("q", jC))[:, 0:n],
                                            rc1[:], MULT)
                if 0 <= jA < nch:
                    E_t = sm_p.tile([128, n], F16, tag="E", bufs=3)
                    st["E", jA] = E_t
                    nc.sync.dma_start(E_t[:], _ap(em_d, jA * ch * W, [[1, n]]))
                    xq_t = sm_p.tile([2 * CIN, n], F32R, tag="xq", bufs=3)
                    st["xq", jA] = xq_t
                    nc.sync.dma_start(
                        xq_t[:], _ap(xp6_d, (jA * ch + PAD) * WP + PAD,
                                     [[WP, ch], [1, W]]))
                # --- Pool: f(jC) late (qp just produced by DVE) ---
                if 0 <= jC < nch:
                    f_t = kk_p.tile([128, KK * n], F16, tag="f", bufs=3)
                    st["f", jC] = f_t
                    nc.gpsimd.tensor_tensor(
                        _ap(f_t[:], 0, [[n, KK], [1, n]]),
                        _ap(st["e1", jC][:], 0, [[n, KK], [1, n]]),
                        _ap(st.pop(("qp", jC))[:], 0, [[0, KK], [1, n]]), MULT)
                    st.pop(("e1", jC))
                # --- DVE tail: out(jE) = r3 * 1/r2; SP: out DMA ---
                if 0 <= jE < nch:
                    rc2 = sm_p.tile([128, n], F32, tag="rc2")
                    nc.vector.reciprocal_approx_fast(
                        rc2[:], st.pop(("r2", jE))[:, 0:n])
                    out_t = sm_p.tile([128, n], F32, tag="out", bufs=3)
                    nc.vector.tensor_tensor(out_t[:],
                                            st.pop(("r3", jE))[:, 0:n],
                                            rc2[:], MULT)
                    for half in (0, 1):
                        nc.sync.dma_start(
                            _ap(out_d, (HH * half + jE * ch) * W, [[1, n]]),
                            out_t[C * half:C * (half + 1), :])
                # --- tail: map-piece copies (DVE/ACT) + v2 (DVE/Pool) ---
                for pi in emit_at.get(cyc, []):
                    piece_copy(pi)


_compiled_nc = None


def _get_nc():
    global _compiled_nc
    if _compiled_nc is None:
        nc = bacc.Bacc("TRN2", target_bir_lowering=False, debug=False,
                       num_devices=N_CORES)
        build_kernel(nc)
        nc.compile()
        _compiled_nc = nc
    return _compiled_nc


def _shard_inputs(x, q_w, k_w, v_w, emb_a, emb_b, emb_mix):
    xp = np.pad(x.astype(np.float32), ((0, 0), (0, 0), (PAD, PAD), (PAD, PAD)))
    # [B, 6, MAPC]: rows 0..2 = ci over padded rows 0..66 (half 0),
    #               rows 3..5 = ci over padded rows 64..130 (half 1)
    xp6 = np.concatenate([xp[:, :, 0:MAPR, :].reshape(B, CIN, MAPC),
                          xp[:, :, HH:HH + MAPR, :].reshape(B, CIN, MAPC)],
                         axis=1)

    def w6(wT):
        full = np.zeros((2 * CIN, 128), np.float32)
        full[0:CIN, 0:C] = wT
        full[CIN:2 * CIN, C:128] = wT
        return np.ascontiguousarray(full)

    # E = (ea + eb) * mix, fused host-side like the padding/packing prep.
    E = (emb_a[:, None, :] + emb_b[:, :, None]) * emb_mix      # [C, H, W]
    em2 = E.reshape(C, 2, HH, W).transpose(1, 0, 2, 3).reshape(128, HH * W)
    common = {
        "w6_q": w6(q_w.T), "w6_k": w6(k_w.T), "w6_v": w6(v_w.T),
        "em2": np.ascontiguousarray(em2.astype(np.float16)),
    }
    return [dict(common, xp6=np.ascontiguousarray(xp6[b]))
            for b in range(B)]


def kernel(x, q_w, k_w, v_w, emb_a, emb_b, emb_mix):
    x, q_w, k_w, v_w, emb_a, emb_b, emb_mix = (
        np.asarray(a, dtype=np.float32)
        for a in (x, q_w, k_w, v_w, emb_a, emb_b, emb_mix))
    nc = _get_nc()
    in_maps = _shard_inputs(x, q_w, k_w, v_w, emb_a, emb_b, emb_mix)
    res = bass_utils.run_bass_kernel_spmd(nc, in_maps, list(range(N_CORES)))
    out = np.stack([res.results[b]["out"].reshape(C, H, W) for b in range(B)])
    return out.astype(np.float32)



# revision 57
# speedup vs baseline: 1.0029x; 1.0029x over previous
"""Trainium2 Bass kernel for nn_AttentionStem (sparse local attention stem).

Math per output element (b, c, h, w), window kk = (di, dj) in 4x4, PAD=2:
  E[c,h,w]   = (emb_a[c,w] + emb_b[c,h]) * emb_mix[c,h,w]
  e1_kk      = exp(v_kk^2 * E)                  (softmax-1 numerator)
  q'         = q / sum_kk(e1)                   (fold softmax-1 denom into q)
  e2_kk      = exp(q' * k_kk * e1_kk)           (softmax-2 numerator)
  out        = sum_kk(e2 * v_kk) / sum_kk(e2)

Sharding: pure data parallel, one batch element per NeuronCore (8 cores).
E is folded on the host (input prep, like the padding/weight packing).

Layout per core: SBUF partition p = 64*half + c (half = h<64 ? 0 : 1), free
dims stream (h, w).  k/v/v2 maps are built once, full-size ([128, 67*132]
fp16), from a 6-partition stacked padded input (both halves' rows), so each
1x1-conv piece is a single matmul; pieces are emitted just-in-time inside
the chunk pipeline so their PSUM->SBUF copies ride in engine slack.

The 32 row-chunks (ch=2 rows per half, n=512 positions) run through an
explicitly software-pipelined schedule — engines execute their instruction
streams in order, so stages of chunk j are emitted across cycles j..j+5
(t1/e1 @ j+1; r1/q/qp/f @ j+2; s2/e2 @ j+4; m2/r2/r3/out @ j+5) and ordered
within each cycle so every queue head waits only on >=1-cycle-old inputs.
Steady state is paced by the Activation engine running the two exps
back-to-back (~7.2us per cycle).  Engine balance (measured cost-model):
  ACT  (0.83 ns/el): e1 = exp(t1), e2 = exp(s2)            ~231 us
  DVE  (0.52 ns/el 2x fp16): t1/s2 windows, qp, out,
        reciprocals, most k/v PSUM->fp16 copies            ~223 us
  Pool (0.83 ns/el): f = e1*q' (kk-broadcast), m2 window   ~222 us
  PE   (0.42 ns/col): convs + 3 sum_kk chains of 16
        PSUM-accumulating identity matmuls (exact fp32)    ~177 us
"""
import sys, os
for _p in ("/opt/trn_rl_repo", "/root/.axon_site/_ro/trn_rl_repo"):
    if os.path.isdir(_p) and _p not in sys.path:
        sys.path.insert(0, _p)

from contextlib import ExitStack, nullcontext as _nullcm
import numpy as np

import concourse.bass as bass
import concourse.bacc as bacc
import concourse.tile as tile
from concourse import mybir
import concourse.bass_utils as bass_utils
from concourse.bass_types import AP
from concourse import masks

N_CORES = 8
B, CIN, H, W = 8, 3, 128, 128
C = 64
K, PAD, KK = 4, 2, 16
HH = H // 2                 # rows per half (64)
WP = W + 2 * PAD            # 132
MAPR = HH + K - 1           # map rows kept per half (67)
MAPC = MAPR * WP            # map cols per partition (8844)
CH = 2                      # h-rows per half per chunk
PIECE = 492                 # conv piece (<=512 psum cols), ceil(8844/18)

F32 = mybir.dt.float32
BF16 = mybir.dt.bfloat16
F16 = mybir.dt.float16
F32R = mybir.dt.float32r
MULT = mybir.AluOpType.mult
EXP = mybir.ActivationFunctionType.Exp


def _ap(base: AP, offset: int, dims):
    """Build a custom free-dim AP on a tile/dram AP, keeping its partition dim."""
    return AP(tensor=base.tensor, offset=base.offset + offset,
              ap=[list(base.ap[0])] + [list(d) for d in dims])


def build_kernel(nc, ch: int = CH, cfg=None, reps: int = 0):
    """reps>0 wraps the whole body in a hardware loop (for benchmarking)."""
    n = ch * W                      # spatial elems per partition per chunk
    nch = HH // ch                  # chunks

    xp6_d = nc.dram_tensor("xp6", [2 * CIN, MAPC], F32R, kind="ExternalInput").ap()
    w6_d = {nm: nc.dram_tensor(f"w6_{nm}", [2 * CIN, 128], F32R,
                               kind="ExternalInput").ap()
            for nm in ("q", "k", "v")}
    em_d = nc.dram_tensor("em2", [128, HH * W], F16, kind="ExternalInput").ap()
    out_d = nc.dram_tensor("out", [C, H * W], F32, kind="ExternalOutput").ap()

    with tile.TileContext(nc) as tc, ExitStack() as ctx:
        const = ctx.enter_context(tc.tile_pool(name="const", bufs=1))
        xp_p = ctx.enter_context(tc.tile_pool(name="xp", bufs=3))
        map_p = ctx.enter_context(tc.tile_pool(name="maps", bufs=1))
        kk_p = ctx.enter_context(tc.tile_pool(name="kk", bufs=6))
        sm_p = ctx.enter_context(tc.tile_pool(name="small", bufs=2))
        ps_mm = ctx.enter_context(tc.tile_pool(name="psmm", bufs=2, space="PSUM"))
        ps_pp = ctx.enter_context(tc.tile_pool(name="pspp", bufs=3, space="PSUM"))
        ps_r1 = ctx.enter_context(tc.tile_pool(name="psr1", bufs=1, space="PSUM"))
        ps_r2 = ctx.enter_context(tc.tile_pool(name="psr2", bufs=1, space="PSUM"))
        ps_r3 = ctx.enter_context(tc.tile_pool(name="psr3", bufs=1, space="PSUM"))

        # ---- constants ----
        w_t = {}
        for nm, d in w6_d.items():
            wt = const.tile([2 * CIN, 128], F32R, tag=f"w{nm}")
            nc.sync.dma_start(wt[:], d[:])
            w_t[nm] = wt
        ident = const.tile([128, 128], F32, tag="ident")
        masks.make_identity(nc, ident[:])
        id_bf = const.tile([128, 128], BF16, tag="idbf")
        nc.vector.tensor_copy(id_bf[:], ident[:])
        id_f16 = const.tile([128, 128], F16, tag="idf16")
        nc.vector.tensor_copy(id_f16[:], ident[:])

        loop_cm = tc.For_i(0, reps, 1) if reps else _nullcm()
        with loop_cm:
            # ==== k/v/v2 full maps; pieces emitted inside the early cycles ====
            k_map = map_p.tile([128, MAPC], F16, tag="kmap")
            v_map = map_p.tile([128, MAPC], F16, tag="vmap")
            v2_map = map_p.tile([128, MAPC], F16, tag="v2map")

            pieces = list(range(0, MAPC, PIECE))
            pst = {}

            def piece_mm(pi):
                pc = pieces[pi]
                pw = min(PIECE, MAPC - pc)
                xt = xp_p.tile([2 * CIN, PIECE], F32R, tag="xp")
                nc.sync.dma_start(xt[:, 0:pw], _ap(xp6_d, pc, [[1, pw]]))
                for nm in ("k", "v"):
                    pt = ps_pp.tile([128, 512], F32, tag="pp")
                    pst[nm, pi] = pt
                    nc.tensor.matmul(pt[:, 0:pw], w_t[nm][:], xt[:, 0:pw],
                                     start=True, stop=True)

            def piece_copy(pi):
                pc = pieces[pi]
                pw = min(PIECE, MAPC - pc)
                for nm, mp in (("k", k_map), ("v", v_map)):
                    pt = pst.pop((nm, pi))
                    nc.vector.tensor_copy(mp[:, pc:pc + pw], pt[:, 0:pw])
                if pi % 2 == 0:
                    nc.vector.tensor_tensor(v2_map[:, pc:pc + pw],
                                            v_map[:, pc:pc + pw],
                                            v_map[:, pc:pc + pw], MULT)
                else:
                    nc.gpsimd.tensor_tensor(v2_map[:, pc:pc + pw],
                                            v_map[:, pc:pc + pw],
                                            v_map[:, pc:pc + pw], MULT)

            # Just-in-time piece schedule: piece pi's first consumer is
            # t1(j*) at cycle j*+1; emit it one cycle earlier.
            emit_at = {}
            for pi, pc in enumerate(pieces):
                r = pc // WP
                jstar = max(0, -(-(r - 5) // 2))
                emit_at.setdefault(max(0, jstar - 1), []).append(pi)

            # ==== phase B: attention chunks, software-pipelined ====
            # Stage offsets (chunk j): DMA/A/E @ cycle j; t1/e1 @ j+1;
            # r1/q-mm/rc1/qp/f @ j+2; s2/e2 @ j+4; m2/r2/r3/rc2/out @ j+5.
            # Per-engine emission order within a cycle keeps every queue head
            # on a >=1-cycle-old dependency (in-order engines never stall on
            # same-cycle work that sits behind them).
            st = {}             # (name, j) -> tile AP

            def win(mp, j, di):
                return _ap(mp[:], (j * ch + di) * WP, [[1, K], [WP, ch], [1, W]])

            def kkslice(t, di):
                return _ap(t[:], di * K * n, [[n, K], [W, ch], [1, W]])

            def reduce(src, acc, idt):
                for kk in range(KK):
                    nc.tensor.matmul(acc[:, 0:n], idt[:],
                                     src[:, kk * n:(kk + 1) * n],
                                     start=(kk == 0), stop=(kk == KK - 1))

            for cyc in range(nch + 6):
                jA = cyc            # DMA, A, E
                jB = cyc - 1        # t1, e1
                jC = cyc - 2        # r1, q-mm, rc1, qp, f
                jD = cyc - 4        # s2, e2
                jE = cyc - 5        # m2, r2, r3, rc2, out

                # --- SP/PE: JIT map-piece DMA + matmuls first ---
                for pi in emit_at.get(cyc, []):
                    piece_mm(pi)
                # --- Pool: m2(jE) (last chunk split with DVE to cut drain) ---
                if 0 <= jE < nch:
                    e2 = st["e2", jE]
                    m2 = kk_p.tile([128, KK * n], F16, tag="m2", bufs=2)
                    st["m2", jE] = m2
                    last = jE == nch - 1
                    for di in range(K):
                        eng = nc.vector if (last and di < 2) else nc.gpsimd
                        eng.tensor_tensor(
                            kkslice(m2, di), kkslice(e2, di),
                            win(v_map, jE, di), MULT)
                # --- DVE: s2(jD) first (f is 2 cycles old) ---
                if 0 <= jD < nch:
                    f_t = st.pop(("f", jD))
                    s2 = kk_p.tile([128, KK * n], F16, tag="s2", bufs=2)
                    st["s2", jD] = s2
                    for di in range(K):
                        nc.vector.tensor_tensor(
                            kkslice(s2, di), kkslice(f_t, di),
                            win(k_map, jD, di), MULT)
                # --- PE: r1(jC) first, then q-mm(jC), r2/r3(jE) ---
                if 0 <= jC < nch:
                    r1 = ps_r1.tile([128, 512], F32, tag="r1")
                    st["r1", jC] = r1
                    reduce(st["e1", jC], r1, id_bf)
                    q_ps = ps_mm.tile([128, 512], F32, tag="mm")
                    st["q", jC] = q_ps
                    nc.tensor.matmul(q_ps[:, 0:n], w_t["q"][:],
                                     st.pop(("xq", jC))[:],
                                     start=True, stop=True)
                # --- ACT: e2(jD) then e1(jB) ---
                if 0 <= jD < nch:
                    e2 = kk_p.tile([128, KK * n], F16, tag="e2", bufs=3)
                    st["e2", jD] = e2
                    nc.scalar.activation(e2[:], st.pop(("s2", jD))[:], EXP)
                # --- DVE: t1(jB) ---
                if 0 <= jB < nch:
                    E_t = st.pop(("E", jB))
                    t1 = kk_p.tile([128, KK * n], F16, tag="t1", bufs=2)
                    st["t1", jB] = t1
                    for di in range(K):
                        nc.vector.tensor_tensor(
                            kkslice(t1, di), win(v2_map, jB, di),
                            _ap(E_t[:], 0, [[0, K], [W, ch], [1, W]]), MULT)
                if 0 <= jB < nch:
                    e1 = kk_p.tile([128, KK * n], BF16, tag="e1", bufs=3)
                    st["e1", jB] = e1
                    t1 = st.pop(("t1", jB))
                    if jB == 0:     # split so the first exp starts earlier
                        h = KK * n // 2
                        nc.scalar.activation(e1[:, 0:h], t1[:, 0:h], EXP)
                        nc.scalar.activation(e1[:, h:], t1[:, h:], EXP)
                    else:
                        nc.scalar.activation(e1[:], t1[:], EXP)
                # --- PE: r2/r3(jE) ---
                if 0 <= jE < nch:
                    r2 = ps_r2.tile([128, 512], F32, tag="r2")
                    st["r2", jE] = r2
                    reduce(st.pop(("e2", jE)), r2, id_f16)
                    r3 = ps_r3.tile([128, 512], F32, tag="r3")
                    st["r3", jE] = r3
                    reduce(st.pop(("m2", jE)), r3, id_f16)
                # --- DVE: qp(jC) = q * 1/r1, then smalls(jA), then out(jE) ---
                if 0 <= jC < nch:
                    rc1 = sm_p.tile([128, n], F32, tag="rc1")
                    nc.vector.reciprocal_approx_fast(
                        rc1[:], st.pop(("r1", jC))[:, 0:n])
                    qp = sm_p.tile([128, n], BF16, tag="qp")
                    st["qp", jC] = qp
                    nc.vector.tensor_tensor(qp[:], st.pop(("q", jC))[:, 0:n],
                                            rc1[:], MULT)
                if 0 <= jA < nch:
                    E_t = sm_p.tile([128, n], F16, tag="E", bufs=3)
                    st["E", jA] = E_t
                    nc.sync.dma_start(E_t[:], _ap(em_d, jA * ch * W, [[1, n]]))
                    xq_t = sm_p.tile([2 * CIN, n], F32R, tag="xq", bufs=3)
                    st["xq", jA] = xq_t
                    nc.sync.dma_start(
                        xq_t[:], _ap(xp6_d, (jA * ch + PAD) * WP + PAD,
                                     [[WP, ch], [1, W]]))
                # --- Pool: f(jC) late (qp just produced by DVE) ---
                if 0 <= jC < nch:
                    f_t = kk_p.tile([128, KK * n], F16, tag="f", bufs=3)
                    st["f", jC] = f_t
                    nc.gpsimd.tensor_tensor(
                        _ap(f_t[:], 0, [[n, KK], [1, n]]),
                        _ap(st["e1", jC][:], 0, [[n, KK], [1, n]]),
                        _ap(st.pop(("qp", jC))[:], 0, [[0, KK], [1, n]]), MULT)
                    st.pop(("e1", jC))
                # --- DVE tail: out(jE) = r3 * 1/r2; SP: out DMA ---
                if 0 <= jE < nch:
                    rc2 = sm_p.tile([128, n], F32, tag="rc2")
                    nc.vector.reciprocal_approx_fast(
                        rc2[:], st.pop(("r2", jE))[:, 0:n])
                    out_t = sm_p.tile([128, n], F32, tag="out", bufs=3)
                    nc.vector.tensor_tensor(out_t[:],
                                            st.pop(("r3", jE))[:, 0:n],
                                            rc2[:], MULT)
                    for half in (0, 1):
                        nc.sync.dma_start(
                            _ap(out_d, (HH * half + jE * ch) * W, [[1, n]]),
                            out_t[C * half:C * (half + 1), :])
                # --- tail: map-piece copies (DVE/ACT) + v2 (DVE/Pool) ---
                for pi in emit_at.get(cyc, []):
                    piece_copy(pi)


_compiled_nc = None


def _get_nc():
    global _compiled_nc
    if _compiled_nc is None:
        nc = bacc.Bacc("TRN2", target_bir_lowering=False, debug=False,
                       num_devices=N_CORES)
        build_kernel(nc)
        nc.compile()
        _compiled_nc = nc
    return _compiled_nc


def _shard_inputs(x, q_w, k_w, v_w, emb_a, emb_b, emb_mix):
    xp = np.pad(x.astype(np.float32), ((0, 0), (0, 0), (PAD, PAD), (PAD, PAD)))
    # [B, 6, MAPC]: rows 0..2 = ci over padded rows 0..66 (half 0),
    #               rows 3..5 = ci over padded rows 64..130 (half 1)
    xp6 = np.concatenate([xp[:, :, 0:MAPR, :].reshape(B, CIN, MAPC),
                          xp[:, :, HH:HH + MAPR, :].reshape(B, CIN, MAPC)],
                         axis=1)

    def w6(wT):
        full = np.zeros((2 * CIN, 128), np.float32)
        full[0:CIN, 0:C] = wT
        full[CIN:2 * CIN, C:128] = wT
        return np.ascontiguousarray(full)

    # E = (ea + eb) * mix, fused host-side like the padding/packing prep.
    E = (emb_a[:, None, :] + emb_b[:, :, None]) * emb_mix      # [C, H, W]
    em2 = E.reshape(C, 2, HH, W).transpose(1, 0, 2, 3).reshape(128, HH * W)
    common = {
        "w6_q": w6(q_w.T), "w6_k": w6(k_w.T), "w6_v": w6(v_w.T),
        "em2": np.ascontiguousarray(em2.astype(np.float16)),
    }
    return [dict(common, xp6=np.ascontiguousarray(xp6[b]))
            for b in range(B)]


def kernel(x, q_w, k_w, v_w, emb_a, emb_b, emb_mix):
    x, q_w, k_w, v_w, emb_a, emb_b, emb_mix = (
        np.asarray(a, dtype=np.float32)
        for a in (x, q_w, k_w, v_w, emb_a, emb_b, emb_mix))
    nc = _get_nc()
    in_maps = _shard_inputs(x, q_w, k_w, v_w, emb_a, emb_b, emb_mix)
    res = bass_utils.run_bass_kernel_spmd(nc, in_maps, list(range(N_CORES)))
    out = np.stack([res.results[b]["out"].reshape(C, H, W) for b in range(B)])
    return out.astype(np.float32)



# revision 59
# speedup vs baseline: 1.0052x; 1.0023x over previous
"""Trainium2 Bass kernel for nn_AttentionStem (sparse local attention stem).

Math per output element (b, c, h, w), window kk = (di, dj) in 4x4, PAD=2:
  E[c,h,w]   = (emb_a[c,w] + emb_b[c,h]) * emb_mix[c,h,w]
  e1_kk      = exp(v_kk^2 * E)                  (softmax-1 numerator)
  q'         = q / sum_kk(e1)                   (fold softmax-1 denom into q)
  e2_kk      = exp(q' * k_kk * e1_kk)           (softmax-2 numerator)
  out        = sum_kk(e2 * v_kk) / sum_kk(e2)

Sharding: pure data parallel, one batch element per NeuronCore (8 cores).
E is folded on the host (input prep, like the padding/weight packing).

Layout per core: SBUF partition p = 64*half + c (half = h<64 ? 0 : 1), free
dims stream (h, w).  k/v/v2 maps are built once, full-size ([128, 67*132]
fp16), from a 6-partition stacked padded input (both halves' rows), so each
1x1-conv piece is a single matmul; pieces are emitted just-in-time inside
the chunk pipeline so their PSUM->SBUF copies ride in engine slack.

The 32 row-chunks (ch=2 rows per half, n=512 positions) run through an
explicitly software-pipelined schedule — engines execute their instruction
streams in order, so stages of chunk j are emitted across cycles j..j+5
(t1/e1 @ j+1; r1/q/qp/f @ j+2; s2/e2 @ j+4; m2/r2/r3/out @ j+5) and ordered
within each cycle so every queue head waits only on >=1-cycle-old inputs.
Steady state is paced by the Activation engine running the two exps
back-to-back (~7.2us per cycle).  Engine balance (measured cost-model):
  ACT  (0.83 ns/el): e1 = exp(t1), e2 = exp(s2)            ~231 us
  DVE  (0.52 ns/el 2x fp16): t1/s2 windows, qp, out,
        reciprocals, most k/v PSUM->fp16 copies            ~223 us
  Pool (0.83 ns/el): f = e1*q' (kk-broadcast), m2 window   ~222 us
  PE   (0.42 ns/col): convs + 3 sum_kk chains of 16
        PSUM-accumulating identity matmuls (exact fp32)    ~177 us
"""
import sys, os
for _p in ("/opt/trn_rl_repo", "/root/.axon_site/_ro/trn_rl_repo"):
    if os.path.isdir(_p) and _p not in sys.path:
        sys.path.insert(0, _p)

from contextlib import ExitStack, nullcontext as _nullcm
import numpy as np

import concourse.bass as bass
import concourse.bacc as bacc
import concourse.tile as tile
from concourse import mybir
import concourse.bass_utils as bass_utils
from concourse.bass_types import AP
from concourse import masks

N_CORES = 8
B, CIN, H, W = 8, 3, 128, 128
C = 64
K, PAD, KK = 4, 2, 16
HH = H // 2                 # rows per half (64)
WP = W + 2 * PAD            # 132
MAPR = HH + K - 1           # map rows kept per half (67)
MAPC = MAPR * WP            # map cols per partition (8844)
CH = 2                      # h-rows per half per chunk
PIECE = 492                 # conv piece (<=512 psum cols), ceil(8844/18)

F32 = mybir.dt.float32
BF16 = mybir.dt.bfloat16
F16 = mybir.dt.float16
F32R = mybir.dt.float32r
MULT = mybir.AluOpType.mult
EXP = mybir.ActivationFunctionType.Exp


def _ap(base: AP, offset: int, dims):
    """Build a custom free-dim AP on a tile/dram AP, keeping its partition dim."""
    return AP(tensor=base.tensor, offset=base.offset + offset,
              ap=[list(base.ap[0])] + [list(d) for d in dims])


def build_kernel(nc, ch: int = CH, cfg=None, reps: int = 0):
    """reps>0 wraps the whole body in a hardware loop (for benchmarking)."""
    n = ch * W                      # spatial elems per partition per chunk
    nch = HH // ch                  # chunks

    xp6_d = nc.dram_tensor("xp6", [2 * CIN, MAPC], F32R, kind="ExternalInput").ap()
    w6_d = {nm: nc.dram_tensor(f"w6_{nm}", [2 * CIN, 128], F32R,
                               kind="ExternalInput").ap()
            for nm in ("k", "v", "q")}
    em_d = nc.dram_tensor("em2", [128, HH * W], F16, kind="ExternalInput").ap()
    out_d = nc.dram_tensor("out", [C, H * W], F32, kind="ExternalOutput").ap()

    with tile.TileContext(nc) as tc, ExitStack() as ctx:
        const = ctx.enter_context(tc.tile_pool(name="const", bufs=1))
        xp_p = ctx.enter_context(tc.tile_pool(name="xp", bufs=3))
        map_p = ctx.enter_context(tc.tile_pool(name="maps", bufs=1))
        kk_p = ctx.enter_context(tc.tile_pool(name="kk", bufs=6))
        sm_p = ctx.enter_context(tc.tile_pool(name="small", bufs=2))
        ps_mm = ctx.enter_context(tc.tile_pool(name="psmm", bufs=2, space="PSUM"))
        ps_pp = ctx.enter_context(tc.tile_pool(name="pspp", bufs=3, space="PSUM"))
        ps_r1 = ctx.enter_context(tc.tile_pool(name="psr1", bufs=1, space="PSUM"))
        ps_r2 = ctx.enter_context(tc.tile_pool(name="psr2", bufs=1, space="PSUM"))
        ps_r3 = ctx.enter_context(tc.tile_pool(name="psr3", bufs=1, space="PSUM"))

        # ---- constants ----
        w_t = {}
        for nm, d in w6_d.items():
            wt = const.tile([2 * CIN, 128], F32R, tag=f"w{nm}")
            nc.sync.dma_start(wt[:], d[:])
            w_t[nm] = wt
        ident = const.tile([128, 128], F32, tag="ident")
        masks.make_identity(nc, ident[:])
        id_bf = const.tile([128, 128], BF16, tag="idbf")
        nc.vector.tensor_copy(id_bf[:], ident[:])
        id_f16 = const.tile([128, 128], F16, tag="idf16")
        nc.vector.tensor_copy(id_f16[:], ident[:])

        loop_cm = tc.For_i(0, reps, 1) if reps else _nullcm()
        with loop_cm:
            # ==== k/v/v2 full maps; pieces emitted inside the early cycles ====
            k_map = map_p.tile([128, MAPC], F16, tag="kmap")
            v_map = map_p.tile([128, MAPC], F16, tag="vmap")
            v2_map = map_p.tile([128, MAPC], F16, tag="v2map")

            pieces = list(range(0, MAPC, PIECE))
            pst = {}

            def piece_mm(pi):
                pc = pieces[pi]
                pw = min(PIECE, MAPC - pc)
                xt = xp_p.tile([2 * CIN, PIECE], F32R, tag="xp")
                nc.sync.dma_start(xt[:, 0:pw], _ap(xp6_d, pc, [[1, pw]]))
                for nm in ("k", "v"):
                    pt = ps_pp.tile([128, 512], F32, tag="pp")
                    pst[nm, pi] = pt
                    nc.tensor.matmul(pt[:, 0:pw], w_t[nm][:], xt[:, 0:pw],
                                     start=True, stop=True)

            def piece_copy(pi):
                pc = pieces[pi]
                pw = min(PIECE, MAPC - pc)
                for nm, mp in (("k", k_map), ("v", v_map)):
                    pt = pst.pop((nm, pi))
                    nc.vector.tensor_copy(mp[:, pc:pc + pw], pt[:, 0:pw])
                if pi % 2 == 0:
                    nc.vector.tensor_tensor(v2_map[:, pc:pc + pw],
                                            v_map[:, pc:pc + pw],
                                            v_map[:, pc:pc + pw], MULT)
                else:
                    nc.gpsimd.tensor_tensor(v2_map[:, pc:pc + pw],
                                            v_map[:, pc:pc + pw],
                                            v_map[:, pc:pc + pw], MULT)

            # Just-in-time piece schedule: piece pi's first consumer is
            # t1(j*) at cycle j*+1; emit it one cycle earlier.
            emit_at = {}
            for pi, pc in enumerate(pieces):
                r = pc // WP
                jstar = max(0, -(-(r - 5) // 2))
                emit_at.setdefault(max(0, jstar - 1), []).append(pi)

            # ==== phase B: attention chunks, software-pipelined ====
            # Stage offsets (chunk j): DMA/A/E @ cycle j; t1/e1 @ j+1;
            # r1/q-mm/rc1/qp/f @ j+2; s2/e2 @ j+4; m2/r2/r3/rc2/out @ j+5.
            # Per-engine emission order within a cycle keeps every queue head
            # on a >=1-cycle-old dependency (in-order engines never stall on
            # same-cycle work that sits behind them).
            st = {}             # (name, j) -> tile AP

            def win(mp, j, di):
                return _ap(mp[:], (j * ch + di) * WP, [[1, K], [WP, ch], [1, W]])

            def kkslice(t, di):
                return _ap(t[:], di * K * n, [[n, K], [W, ch], [1, W]])

            def reduce(src, acc, idt):
                for kk in range(KK):
                    nc.tensor.matmul(acc[:, 0:n], idt[:],
                                     src[:, kk * n:(kk + 1) * n],
                                     start=(kk == 0), stop=(kk == KK - 1))

            for cyc in range(nch + 6):
                jA = cyc            # DMA, A, E
                jB = cyc - 1        # t1, e1
                jC = cyc - 2        # r1, q-mm, rc1, qp, f
                jD = cyc - 4        # s2, e2
                jE = cyc - 5        # m2, r2, r3, rc2, out

                # --- SP/PE: JIT map-piece DMA + matmuls first ---
                for pi in emit_at.get(cyc, []):
                    piece_mm(pi)
                # --- Pool: m2(jE) (last chunk split with DVE to cut drain) ---
                if 0 <= jE < nch:
                    e2 = st["e2", jE]
                    m2 = kk_p.tile([128, KK * n], F16, tag="m2", bufs=2)
                    st["m2", jE] = m2
                    last = jE == nch - 1
                    for di in range(K):
                        eng = nc.vector if (last and di < 2) else nc.gpsimd
                        eng.tensor_tensor(
                            kkslice(m2, di), kkslice(e2, di),
                            win(v_map, jE, di), MULT)
                # --- DVE: s2(jD) first (f is 2 cycles old) ---
                if 0 <= jD < nch:
                    f_t = st.pop(("f", jD))
                    s2 = kk_p.tile([128, KK * n], F16, tag="s2", bufs=2)
                    st["s2", jD] = s2
                    for di in range(K):
                        nc.vector.tensor_tensor(
                            kkslice(s2, di), kkslice(f_t, di),
                            win(k_map, jD, di), MULT)
                # --- PE: r1(jC) first, then q-mm(jC), r2/r3(jE) ---
                if 0 <= jC < nch:
                    r1 = ps_r1.tile([128, 512], F32, tag="r1")
                    st["r1", jC] = r1
                    reduce(st["e1", jC], r1, id_bf)
                    q_ps = ps_mm.tile([128, 512], F32, tag="mm")
                    st["q", jC] = q_ps
                    nc.tensor.matmul(q_ps[:, 0:n], w_t["q"][:],
                                     st.pop(("xq", jC))[:],
                                     start=True, stop=True)
                # --- ACT: e2(jD) then e1(jB) ---
                if 0 <= jD < nch:
                    e2 = kk_p.tile([128, KK * n], F16, tag="e2", bufs=3)
                    st["e2", jD] = e2
                    s2 = st.pop(("s2", jD))
                    if jD == nch - 1:   # split so the tail m2 starts earlier
                        h = KK * n // 2
                        nc.scalar.activation(e2[:, 0:h], s2[:, 0:h], EXP)
                        nc.scalar.activation(e2[:, h:], s2[:, h:], EXP)
                    else:
                        nc.scalar.activation(e2[:], s2[:], EXP)
                # --- DVE: t1(jB) ---
                if 0 <= jB < nch:
                    E_t = st.pop(("E", jB))
                    t1 = kk_p.tile([128, KK * n], F16, tag="t1", bufs=2)
                    st["t1", jB] = t1
                    for di in range(K):
                        nc.vector.tensor_tensor(
                            kkslice(t1, di), win(v2_map, jB, di),
                            _ap(E_t[:], 0, [[0, K], [W, ch], [1, W]]), MULT)
                if 0 <= jB < nch:
                    e1 = kk_p.tile([128, KK * n], BF16, tag="e1", bufs=3)
                    st["e1", jB] = e1
                    t1 = st.pop(("t1", jB))
                    if jB == 0:     # split so the first exp starts earlier
                        h = KK * n // 2
                        nc.scalar.activation(e1[:, 0:h], t1[:, 0:h], EXP)
                        nc.scalar.activation(e1[:, h:], t1[:, h:], EXP)
                    else:
                        nc.scalar.activation(e1[:], t1[:], EXP)
                # --- PE: r2/r3(jE) ---
                if 0 <= jE < nch:
                    r2 = ps_r2.tile([128, 512], F32, tag="r2")
                    st["r2", jE] = r2
                    reduce(st.pop(("e2", jE)), r2, id_f16)
                    r3 = ps_r3.tile([128, 512], F32, tag="r3")
                    st["r3", jE] = r3
                    reduce(st.pop(("m2", jE)), r3, id_f16)
                # --- DVE: qp(jC) = q * 1/r1, then smalls(jA), then out(jE) ---
                if 0 <= jC < nch:
                    rc1 = sm_p.tile([128, n], F32, tag="rc1")
                    nc.vector.reciprocal_approx_fast(
                        rc1[:], st.pop(("r1", jC))[:, 0:n])
                    qp = sm_p.tile([128, n], BF16, tag="qp")
                    st["qp", jC] = qp
                    nc.vector.tensor_tensor(qp[:], st.pop(("q", jC))[:, 0:n],
                                            rc1[:], MULT)
                if 0 <= jA < nch:
                    E_t = sm_p.tile([128, n], F16, tag="E", bufs=3)
                    st["E", jA] = E_t
                    nc.sync.dma_start(E_t[:], _ap(em_d, jA * ch * W, [[1, n]]))
                    xq_t = sm_p.tile([2 * CIN, n], F32R, tag="xq", bufs=3)
                    st["xq", jA] = xq_t
                    nc.sync.dma_start(
                        xq_t[:], _ap(xp6_d, (jA * ch + PAD) * WP + PAD,
                                     [[WP, ch], [1, W]]))
                # --- Pool: f(jC) late (qp just produced by DVE) ---
                if 0 <= jC < nch:
                    f_t = kk_p.tile([128, KK * n], F16, tag="f", bufs=3)
                    st["f", jC] = f_t
                    nc.gpsimd.tensor_tensor(
                        _ap(f_t[:], 0, [[n, KK], [1, n]]),
                        _ap(st["e1", jC][:], 0, [[n, KK], [1, n]]),
                        _ap(st.pop(("qp", jC))[:], 0, [[0, KK], [1, n]]), MULT)
                    st.pop(("e1", jC))
                # --- DVE tail: out(jE) = r3 * 1/r2; SP: out DMA ---
                if 0 <= jE < nch:
                    rc2 = sm_p.tile([128, n], F32, tag="rc2")
                    nc.vector.reciprocal_approx_fast(
                        rc2[:], st.pop(("r2", jE))[:, 0:n])
                    out_t = sm_p.tile([128, n], F32, tag="out", bufs=3)
                    nc.vector.tensor_tensor(out_t[:],
                                            st.pop(("r3", jE))[:, 0:n],
                                            rc2[:], MULT)
                    for half in (0, 1):
                        nc.sync.dma_start(
                            _ap(out_d, (HH * half + jE * ch) * W, [[1, n]]),
                            out_t[C * half:C * (half + 1), :])
                # --- tail: map-piece copies (DVE/ACT) + v2 (DVE/Pool) ---
                for pi in emit_at.get(cyc, []):
                    piece_copy(pi)


_compiled_nc = None


def _get_nc():
    global _compiled_nc
    if _compiled_nc is None:
        nc = bacc.Bacc("TRN2", target_bir_lowering=False, debug=False,
                       num_devices=N_CORES)
        build_kernel(nc)
        nc.compile()
        _compiled_nc = nc
    return _compiled_nc


def _shard_inputs(x, q_w, k_w, v_w, emb_a, emb_b, emb_mix):
    xp = np.pad(x.astype(np.float32), ((0, 0), (0, 0), (PAD, PAD), (PAD, PAD)))
    # [B, 6, MAPC]: rows 0..2 = ci over padded rows 0..66 (half 0),
    #               rows 3..5 = ci over padded rows 64..130 (half 1)
    xp6 = np.concatenate([xp[:, :, 0:MAPR, :].reshape(B, CIN, MAPC),
                          xp[:, :, HH:HH + MAPR, :].reshape(B, CIN, MAPC)],
                         axis=1)

    def w6(wT):
        full = np.zeros((2 * CIN, 128), np.float32)
        full[0:CIN, 0:C] = wT
        full[CIN:2 * CIN, C:128] = wT
        return np.ascontiguousarray(full)

    # E = (ea + eb) * mix, fused host-side like the padding/packing prep.
    E = (emb_a[:, None, :] + emb_b[:, :, None]) * emb_mix      # [C, H, W]
    em2 = E.reshape(C, 2, HH, W).transpose(1, 0, 2, 3).reshape(128, HH * W)
    common = {
        "w6_q": w6(q_w.T), "w6_k": w6(k_w.T), "w6_v": w6(v_w.T),
        "em2": np.ascontiguousarray(em2.astype(np.float16)),
    }
    return [dict(common, xp6=np.ascontiguousarray(xp6[b]))
            for b in range(B)]


def kernel(x, q_w, k_w, v_w, emb_a, emb_b, emb_mix):
    x, q_w, k_w, v_w, emb_a, emb_b, emb_mix = (
        np.asarray(a, dtype=np.float32)
        for a in (x, q_w, k_w, v_w, emb_a, emb_b, emb_mix))
    nc = _get_nc()
    in_maps = _shard_inputs(x, q_w, k_w, v_w, emb_a, emb_b, emb_mix)
    res = bass_utils.run_bass_kernel_spmd(nc, in_maps, list(range(N_CORES)))
    out = np.stack([res.results[b]["out"].reshape(C, H, W) for b in range(B)])
    return out.astype(np.float32)



# revision 64
# speedup vs baseline: 1.0110x; 1.0058x over previous
"""Trainium2 Bass kernel for nn_AttentionStem (sparse local attention stem).

Math per output element (b, c, h, w), window kk = (di, dj) in 4x4, PAD=2:
  E[c,h,w]   = (emb_a[c,w] + emb_b[c,h]) * emb_mix[c,h,w]
  e1_kk      = exp(v_kk^2 * E)                  (softmax-1 numerator)
  q'         = q / sum_kk(e1)                   (fold softmax-1 denom into q)
  e2_kk      = exp(q' * k_kk * e1_kk)           (softmax-2 numerator)
  out        = sum_kk(e2 * v_kk) / sum_kk(e2)

Sharding: pure data parallel, one batch element per NeuronCore (8 cores).
E is folded on the host (input prep, like the padding/weight packing).

Layout per core: SBUF partition p = 64*half + c (half = h<64 ? 0 : 1), free
dims stream (h, w).  k/v/v2 maps are built once, full-size ([128, 67*132]
fp16), from a 6-partition stacked padded input (both halves' rows), so each
1x1-conv piece is a single matmul; pieces are emitted just-in-time inside
the chunk pipeline so their PSUM->SBUF copies ride in engine slack.

The 32 row-chunks (ch=2 rows per half, n=512 positions) run through an
explicitly software-pipelined schedule — engines execute their instruction
streams in order, so stages of chunk j are emitted across cycles j..j+5
(t1/e1 @ j+1; r1/q/qp/f @ j+2; s2/e2 @ j+4; m2/r2/r3/out @ j+5) and ordered
within each cycle so every queue head waits only on >=1-cycle-old inputs.
Steady state is paced by the Activation engine running the two exps
back-to-back (~7.2us per cycle).  Engine balance (measured cost-model):
  ACT  (0.83 ns/el): e1 = exp(t1), e2 = exp(s2)            ~231 us
  DVE  (0.52 ns/el 2x fp16): t1/s2 windows, qp, out,
        reciprocals, most k/v PSUM->fp16 copies            ~223 us
  Pool (0.83 ns/el): f = e1*q' (kk-broadcast), m2 window   ~222 us
  PE   (0.42 ns/col): convs + 3 sum_kk chains of 16
        PSUM-accumulating identity matmuls (exact fp32)    ~177 us
"""
import sys, os
for _p in ("/opt/trn_rl_repo", "/root/.axon_site/_ro/trn_rl_repo"):
    if os.path.isdir(_p) and _p not in sys.path:
        sys.path.insert(0, _p)

from contextlib import ExitStack, nullcontext as _nullcm
import numpy as np

import concourse.bass as bass
import concourse.bacc as bacc
import concourse.tile as tile
from concourse import mybir
import concourse.bass_utils as bass_utils
from concourse.bass_types import AP
from concourse import masks

N_CORES = 8
B, CIN, H, W = 8, 3, 128, 128
C = 64
K, PAD, KK = 4, 2, 16
HH = H // 2                 # rows per half (64)
WP = W + 2 * PAD            # 132
MAPR = HH + K - 1           # map rows kept per half (67)
MAPC = MAPR * WP            # map cols per partition (8844)
CH = 2                      # h-rows per half per chunk
PIECE = 492                 # conv piece (<=512 psum cols), ceil(8844/18)

F32 = mybir.dt.float32
BF16 = mybir.dt.bfloat16
F16 = mybir.dt.float16
F32R = mybir.dt.float32r
MULT = mybir.AluOpType.mult
EXP = mybir.ActivationFunctionType.Exp


def _ap(base: AP, offset: int, dims):
    """Build a custom free-dim AP on a tile/dram AP, keeping its partition dim."""
    return AP(tensor=base.tensor, offset=base.offset + offset,
              ap=[list(base.ap[0])] + [list(d) for d in dims])


def build_kernel(nc, ch: int = CH, cfg=None, reps: int = 0):
    """reps>0 wraps the whole body in a hardware loop (for benchmarking)."""
    n = ch * W                      # spatial elems per partition per chunk
    nch = HH // ch                  # chunks

    xp6_d = nc.dram_tensor("xp6", [2 * CIN, MAPC], F32R, kind="ExternalInput").ap()
    w6_d = nc.dram_tensor("w6", [2 * CIN, 3 * 128], F32R,
                          kind="ExternalInput").ap()
    em_d = nc.dram_tensor("em2", [128, HH * W], F16, kind="ExternalInput").ap()
    out_d = nc.dram_tensor("out", [C, H * W], F32, kind="ExternalOutput").ap()

    with tile.TileContext(nc) as tc, ExitStack() as ctx:
        const = ctx.enter_context(tc.tile_pool(name="const", bufs=1))
        xp_p = ctx.enter_context(tc.tile_pool(name="xp", bufs=3))
        map_p = ctx.enter_context(tc.tile_pool(name="maps", bufs=1))
        kk_p = ctx.enter_context(tc.tile_pool(name="kk", bufs=6))
        sm_p = ctx.enter_context(tc.tile_pool(name="small", bufs=2))
        ps_mm = ctx.enter_context(tc.tile_pool(name="psmm", bufs=2, space="PSUM"))
        ps_pp = ctx.enter_context(tc.tile_pool(name="pspp", bufs=3, space="PSUM"))
        ps_r1 = ctx.enter_context(tc.tile_pool(name="psr1", bufs=1, space="PSUM"))
        ps_r2 = ctx.enter_context(tc.tile_pool(name="psr2", bufs=1, space="PSUM"))
        ps_r3 = ctx.enter_context(tc.tile_pool(name="psr3", bufs=1, space="PSUM"))

        # ---- constants (k/v/q weights in one DMA) ----
        w_all = const.tile([2 * CIN, 3 * 128], F32R, tag="w6")
        nc.sync.dma_start(w_all[:], w6_d[:])
        w_t = {nm: w_all[:, 128 * i:128 * (i + 1)]
               for i, nm in enumerate(("k", "v", "q"))}
        ident = const.tile([128, 128], F32, tag="ident")
        masks.make_identity(nc, ident[:])
        id_bf = const.tile([128, 128], BF16, tag="idbf")
        nc.vector.tensor_copy(id_bf[:], ident[:])
        id_f16 = const.tile([128, 128], F16, tag="idf16")
        nc.vector.tensor_copy(id_f16[:], ident[:])

        loop_cm = tc.For_i(0, reps, 1) if reps else _nullcm()
        with loop_cm:
            # ==== k/v/v2 full maps; pieces emitted inside the early cycles ====
            k_map = map_p.tile([128, MAPC], F16, tag="kmap")
            v_map = map_p.tile([128, MAPC], F16, tag="vmap")
            v2_map = map_p.tile([128, MAPC], F16, tag="v2map")

            pieces = list(range(0, MAPC, PIECE))
            pst = {}

            def piece_mm(pi):
                pc = pieces[pi]
                pw = min(PIECE, MAPC - pc)
                xt = xp_p.tile([2 * CIN, PIECE], F32R, tag="xp")
                nc.sync.dma_start(xt[:, 0:pw], _ap(xp6_d, pc, [[1, pw]]))
                for nm in ("k", "v"):
                    pt = ps_pp.tile([128, 512], F32, tag="pp")
                    pst[nm, pi] = pt
                    nc.tensor.matmul(pt[:, 0:pw], w_t[nm], xt[:, 0:pw],
                                     start=True, stop=True)

            def piece_copy(pi):
                pc = pieces[pi]
                pw = min(PIECE, MAPC - pc)
                for nm, mp in (("k", k_map), ("v", v_map)):
                    pt = pst.pop((nm, pi))
                    nc.vector.tensor_copy(mp[:, pc:pc + pw], pt[:, 0:pw])
                if pi % 2 == 0:
                    nc.vector.tensor_tensor(v2_map[:, pc:pc + pw],
                                            v_map[:, pc:pc + pw],
                                            v_map[:, pc:pc + pw], MULT)
                else:
                    nc.gpsimd.tensor_tensor(v2_map[:, pc:pc + pw],
                                            v_map[:, pc:pc + pw],
                                            v_map[:, pc:pc + pw], MULT)

            # Just-in-time piece schedule: piece pi's first consumer is
            # t1(j*) at cycle j*+1; emit it one cycle earlier.
            emit_at = {}
            for pi, pc in enumerate(pieces):
                r = pc // WP
                jstar = max(0, -(-(r - 5) // 2))
                emit_at.setdefault(max(0, jstar - 1), []).append(pi)

            # ==== phase B: attention chunks, software-pipelined ====
            # Stage offsets (chunk j): DMA/A/E @ cycle j; t1/e1 @ j+1;
            # r1/q-mm/rc1/qp/f @ j+2; s2/e2 @ j+4; m2/r2/r3/rc2/out @ j+5.
            # Per-engine emission order within a cycle keeps every queue head
            # on a >=1-cycle-old dependency (in-order engines never stall on
            # same-cycle work that sits behind them).
            st = {}             # (name, j) -> tile AP

            def win(mp, j, di):
                return _ap(mp[:], (j * ch + di) * WP, [[1, K], [WP, ch], [1, W]])

            def kkslice(t, di):
                return _ap(t[:], di * K * n, [[n, K], [W, ch], [1, W]])

            def reduce(src, acc, idt):
                for kk in range(KK):
                    nc.tensor.matmul(acc[:, 0:n], idt[:],
                                     src[:, kk * n:(kk + 1) * n],
                                     start=(kk == 0), stop=(kk == KK - 1))

            for cyc in range(nch + 6):
                jA = cyc            # DMA, A, E
                jB = cyc - 1        # t1, e1
                jC = cyc - 2        # r1, q-mm, rc1, qp, f
                jD = cyc - 4        # s2, e2
                jE = cyc - 5        # m2, r2, r3, rc2, out

                # --- SP/PE: JIT map-piece DMA + matmuls first ---
                for pi in emit_at.get(cyc, []):
                    piece_mm(pi)
                # --- Pool: m2(jE) (last chunk split with DVE to cut drain) ---
                if 0 <= jE < nch:
                    e2 = st["e2", jE]
                    m2 = kk_p.tile([128, KK * n], F16, tag="m2", bufs=2)
                    st["m2", jE] = m2
                    last = jE == nch - 1
                    for di in range(K):
                        eng = nc.vector if (last and di < 2) else nc.gpsimd
                        eng.tensor_tensor(
                            kkslice(m2, di), kkslice(e2, di),
                            win(v_map, jE, di), MULT)
                # --- DVE: s2(jD) first (f is 2 cycles old) ---
                if 0 <= jD < nch:
                    f_t = st.pop(("f", jD))
                    s2 = kk_p.tile([128, KK * n], F16, tag="s2", bufs=2)
                    st["s2", jD] = s2
                    for di in range(K):
                        nc.vector.tensor_tensor(
                            kkslice(s2, di), kkslice(f_t, di),
                            win(k_map, jD, di), MULT)
                # --- PE: r1(jC) first, then q-mm(jC), r2/r3(jE) ---
                if 0 <= jC < nch:
                    r1 = ps_r1.tile([128, 512], F32, tag="r1")
                    st["r1", jC] = r1
                    reduce(st["e1", jC], r1, id_bf)
                    q_ps = ps_mm.tile([128, 512], F32, tag="mm")
                    st["q", jC] = q_ps
                    nc.tensor.matmul(q_ps[:, 0:n], w_t["q"],
                                     st.pop(("xq", jC))[:],
                                     start=True, stop=True)
                # --- ACT: e2(jD) then e1(jB) ---
                if 0 <= jD < nch:
                    e2 = kk_p.tile([128, KK * n], F16, tag="e2", bufs=3)
                    st["e2", jD] = e2
                    s2 = st.pop(("s2", jD))
                    if jD == nch - 1:   # split so the tail m2 starts earlier
                        h = KK * n // 2
                        nc.scalar.activation(e2[:, 0:h], s2[:, 0:h], EXP)
                        nc.scalar.activation(e2[:, h:], s2[:, h:], EXP)
                    else:
                        nc.scalar.activation(e2[:], s2[:], EXP)
                # --- DVE: t1(jB) ---
                if 0 <= jB < nch:
                    E_t = st.pop(("E", jB))
                    t1 = kk_p.tile([128, KK * n], F16, tag="t1", bufs=2)
                    st["t1", jB] = t1
                    for di in range(K):
                        nc.vector.tensor_tensor(
                            kkslice(t1, di), win(v2_map, jB, di),
                            _ap(E_t[:], 0, [[0, K], [W, ch], [1, W]]), MULT)
                if 0 <= jB < nch:
                    e1 = kk_p.tile([128, KK * n], BF16, tag="e1", bufs=3)
                    st["e1", jB] = e1
                    t1 = st.pop(("t1", jB))
                    if jB == 0:     # split so the first exp starts earlier
                        h = KK * n // 2
                        nc.scalar.activation(e1[:, 0:h], t1[:, 0:h], EXP)
                        nc.scalar.activation(e1[:, h:], t1[:, h:], EXP)
                    else:
                        nc.scalar.activation(e1[:], t1[:], EXP)
                # --- PE: r2/r3(jE) ---
                if 0 <= jE < nch:
                    r2 = ps_r2.tile([128, 512], F32, tag="r2")
                    st["r2", jE] = r2
                    reduce(st.pop(("e2", jE)), r2, id_f16)
                    r3 = ps_r3.tile([128, 512], F32, tag="r3")
                    st["r3", jE] = r3
                    reduce(st.pop(("m2", jE)), r3, id_f16)
                # --- DVE: qp(jC) = q * 1/r1, then smalls(jA), then out(jE) ---
                if 0 <= jC < nch:
                    rc1 = sm_p.tile([128, n], F32, tag="rc1")
                    nc.vector.reciprocal_approx_fast(
                        rc1[:], st.pop(("r1", jC))[:, 0:n])
                    qp = sm_p.tile([128, n], BF16, tag="qp")
                    st["qp", jC] = qp
                    nc.vector.tensor_tensor(qp[:], st.pop(("q", jC))[:, 0:n],
                                            rc1[:], MULT)
                if 0 <= jA < nch:
                    E_t = sm_p.tile([128, n], F16, tag="E", bufs=3)
                    st["E", jA] = E_t
                    nc.sync.dma_start(E_t[:], _ap(em_d, jA * ch * W, [[1, n]]))
                    xq_t = sm_p.tile([2 * CIN, n], F32R, tag="xq", bufs=3)
                    st["xq", jA] = xq_t
                    nc.sync.dma_start(
                        xq_t[:], _ap(xp6_d, (jA * ch + PAD) * WP + PAD,
                                     [[WP, ch], [1, W]]))
                # --- Pool: f(jC) late (qp just produced by DVE) ---
                if 0 <= jC < nch:
                    f_t = kk_p.tile([128, KK * n], F16, tag="f", bufs=3)
                    st["f", jC] = f_t
                    nc.gpsimd.tensor_tensor(
                        _ap(f_t[:], 0, [[n, KK], [1, n]]),
                        _ap(st["e1", jC][:], 0, [[n, KK], [1, n]]),
                        _ap(st.pop(("qp", jC))[:], 0, [[0, KK], [1, n]]), MULT)
                    st.pop(("e1", jC))
                # --- DVE tail: out(jE) = r3 * 1/r2; SP: out DMA ---
                if 0 <= jE < nch:
                    rc2 = sm_p.tile([128, n], F32, tag="rc2")
                    nc.vector.reciprocal_approx_fast(
                        rc2[:], st.pop(("r2", jE))[:, 0:n])
                    out_t = sm_p.tile([128, n], F32, tag="out", bufs=3)
                    nc.vector.tensor_tensor(out_t[:],
                                            st.pop(("r3", jE))[:, 0:n],
                                            rc2[:], MULT)
                    for half in (0, 1):
                        nc.sync.dma_start(
                            _ap(out_d, (HH * half + jE * ch) * W, [[1, n]]),
                            out_t[C * half:C * (half + 1), :])
                # --- tail: map-piece copies (DVE/ACT) + v2 (DVE/Pool) ---
                for pi in emit_at.get(cyc, []):
                    piece_copy(pi)


_compiled_nc = None


def _get_nc():
    global _compiled_nc
    if _compiled_nc is None:
        nc = bacc.Bacc("TRN2", target_bir_lowering=False, debug=False,
                       num_devices=N_CORES)
        build_kernel(nc)
        nc.compile()
        _compiled_nc = nc
    return _compiled_nc


def _shard_inputs(x, q_w, k_w, v_w, emb_a, emb_b, emb_mix):
    xp = np.pad(x.astype(np.float32), ((0, 0), (0, 0), (PAD, PAD), (PAD, PAD)))
    # [B, 6, MAPC]: rows 0..2 = ci over padded rows 0..66 (half 0),
    #               rows 3..5 = ci over padded rows 64..130 (half 1)
    xp6 = np.concatenate([xp[:, :, 0:MAPR, :].reshape(B, CIN, MAPC),
                          xp[:, :, HH:HH + MAPR, :].reshape(B, CIN, MAPC)],
                         axis=1)

    def w6(wT):
        full = np.zeros((2 * CIN, 128), np.float32)
        full[0:CIN, 0:C] = wT
        full[CIN:2 * CIN, C:128] = wT
        return full

    # E = (ea + eb) * mix, fused host-side like the padding/packing prep.
    E = (emb_a[:, None, :] + emb_b[:, :, None]) * emb_mix      # [C, H, W]
    em2 = E.reshape(C, 2, HH, W).transpose(1, 0, 2, 3).reshape(128, HH * W)
    common = {
        "w6": np.ascontiguousarray(
            np.concatenate([w6(k_w.T), w6(v_w.T), w6(q_w.T)], axis=1)),
        "em2": np.ascontiguousarray(em2.astype(np.float16)),
    }
    return [dict(common, xp6=np.ascontiguousarray(xp6[b]))
            for b in range(B)]


def kernel(x, q_w, k_w, v_w, emb_a, emb_b, emb_mix):
    x, q_w, k_w, v_w, emb_a, emb_b, emb_mix = (
        np.asarray(a, dtype=np.float32)
        for a in (x, q_w, k_w, v_w, emb_a, emb_b, emb_mix))
    nc = _get_nc()
    in_maps = _shard_inputs(x, q_w, k_w, v_w, emb_a, emb_b, emb_mix)
    res = bass_utils.run_bass_kernel_spmd(nc, in_maps, list(range(N_CORES)))
    out = np.stack([res.results[b]["out"].reshape(C, H, W) for b in range(B)])
    return out.astype(np.float32)



# revision 65
# speedup vs baseline: 1.0125x; 1.0015x over previous
"""Trainium2 Bass kernel for nn_AttentionStem (sparse local attention stem).

Math per output element (b, c, h, w), window kk = (di, dj) in 4x4, PAD=2:
  E[c,h,w]   = (emb_a[c,w] + emb_b[c,h]) * emb_mix[c,h,w]
  e1_kk      = exp(v_kk^2 * E)                  (softmax-1 numerator)
  q'         = q / sum_kk(e1)                   (fold softmax-1 denom into q)
  e2_kk      = exp(q' * k_kk * e1_kk)           (softmax-2 numerator)
  out        = sum_kk(e2 * v_kk) / sum_kk(e2)

Sharding: pure data parallel, one batch element per NeuronCore (8 cores).
E is folded on the host (input prep, like the padding/weight packing).

Layout per core: SBUF partition p = 64*half + c (half = h<64 ? 0 : 1), free
dims stream (h, w).  k/v/v2 maps are built once, full-size ([128, 67*132]
fp16), from a 6-partition stacked padded input (both halves' rows), so each
1x1-conv piece is a single matmul; pieces are emitted just-in-time inside
the chunk pipeline so their PSUM->SBUF copies ride in engine slack.

The 32 row-chunks (ch=2 rows per half, n=512 positions) run through an
explicitly software-pipelined schedule — engines execute their instruction
streams in order, so stages of chunk j are emitted across cycles j..j+5
(t1/e1 @ j+1; r1/q/qp/f @ j+2; s2/e2 @ j+4; m2/r2/r3/out @ j+5) and ordered
within each cycle so every queue head waits only on >=1-cycle-old inputs.
Steady state is paced by the Activation engine running the two exps
back-to-back (~7.2us per cycle).  Engine balance (measured cost-model):
  ACT  (0.83 ns/el): e1 = exp(t1), e2 = exp(s2)            ~231 us
  DVE  (0.52 ns/el 2x fp16): t1/s2 windows, qp, out,
        reciprocals, most k/v PSUM->fp16 copies            ~223 us
  Pool (0.83 ns/el): f = e1*q' (kk-broadcast), m2 window   ~222 us
  PE   (0.42 ns/col): convs + 3 sum_kk chains of 16
        PSUM-accumulating identity matmuls (exact fp32)    ~177 us
"""
import sys, os
for _p in ("/opt/trn_rl_repo", "/root/.axon_site/_ro/trn_rl_repo"):
    if os.path.isdir(_p) and _p not in sys.path:
        sys.path.insert(0, _p)

from contextlib import ExitStack, nullcontext as _nullcm
import numpy as np

import concourse.bass as bass
import concourse.bacc as bacc
import concourse.tile as tile
from concourse import mybir
import concourse.bass_utils as bass_utils
from concourse.bass_types import AP
from concourse import masks

N_CORES = 8
B, CIN, H, W = 8, 3, 128, 128
C = 64
K, PAD, KK = 4, 2, 16
HH = H // 2                 # rows per half (64)
WP = W + 2 * PAD            # 132
MAPR = HH + K - 1           # map rows kept per half (67)
MAPC = MAPR * WP            # map cols per partition (8844)
CH = 2                      # h-rows per half per chunk
PIECE = 492                 # conv piece (<=512 psum cols), ceil(8844/18)

F32 = mybir.dt.float32
BF16 = mybir.dt.bfloat16
F16 = mybir.dt.float16
F32R = mybir.dt.float32r
MULT = mybir.AluOpType.mult
EXP = mybir.ActivationFunctionType.Exp


def _ap(base: AP, offset: int, dims):
    """Build a custom free-dim AP on a tile/dram AP, keeping its partition dim."""
    return AP(tensor=base.tensor, offset=base.offset + offset,
              ap=[list(base.ap[0])] + [list(d) for d in dims])


def build_kernel(nc, ch: int = CH, cfg=None, reps: int = 0):
    """reps>0 wraps the whole body in a hardware loop (for benchmarking)."""
    n = ch * W                      # spatial elems per partition per chunk
    nch = HH // ch                  # chunks

    xp6_d = nc.dram_tensor("xp6", [2 * CIN, MAPC], F32R, kind="ExternalInput").ap()
    w6_d = nc.dram_tensor("w6", [2 * CIN, 3 * 128], F32R,
                          kind="ExternalInput").ap()
    em_d = nc.dram_tensor("em2", [128, HH * W], F16, kind="ExternalInput").ap()
    out_d = nc.dram_tensor("out", [C, H * W], F32, kind="ExternalOutput").ap()

    with tile.TileContext(nc) as tc, ExitStack() as ctx:
        const = ctx.enter_context(tc.tile_pool(name="const", bufs=1))
        xp_p = ctx.enter_context(tc.tile_pool(name="xp", bufs=3))
        map_p = ctx.enter_context(tc.tile_pool(name="maps", bufs=1))
        kk_p = ctx.enter_context(tc.tile_pool(name="kk", bufs=6))
        sm_p = ctx.enter_context(tc.tile_pool(name="small", bufs=2))
        ps_mm = ctx.enter_context(tc.tile_pool(name="psmm", bufs=2, space="PSUM"))
        ps_pp = ctx.enter_context(tc.tile_pool(name="pspp", bufs=3, space="PSUM"))
        ps_r1 = ctx.enter_context(tc.tile_pool(name="psr1", bufs=1, space="PSUM"))
        ps_r2 = ctx.enter_context(tc.tile_pool(name="psr2", bufs=1, space="PSUM"))
        ps_r3 = ctx.enter_context(tc.tile_pool(name="psr3", bufs=1, space="PSUM"))

        # ---- constants (k/v/q weights in one DMA) ----
        w_all = const.tile([2 * CIN, 3 * 128], F32R, tag="w6")
        nc.sync.dma_start(w_all[:], w6_d[:])
        w_t = {nm: w_all[:, 128 * i:128 * (i + 1)]
               for i, nm in enumerate(("k", "v", "q"))}
        ident = const.tile([128, 128], F32, tag="ident")
        masks.make_identity(nc, ident[:])
        id_bf = const.tile([128, 128], BF16, tag="idbf")
        nc.vector.tensor_copy(id_bf[:], ident[:])
        id_f16 = const.tile([128, 128], F16, tag="idf16")
        nc.vector.tensor_copy(id_f16[:], ident[:])

        loop_cm = tc.For_i(0, reps, 1) if reps else _nullcm()
        with loop_cm:
            # ==== k/v/v2 full maps; pieces emitted inside the early cycles ====
            k_map = map_p.tile([128, MAPC], F16, tag="kmap")
            v_map = map_p.tile([128, MAPC], F16, tag="vmap")
            v2_map = map_p.tile([128, MAPC], F16, tag="v2map")

            pieces = list(range(0, MAPC, PIECE))
            pst = {}

            def piece_mm(pi):
                pc = pieces[pi]
                pw = min(PIECE, MAPC - pc)
                xt = xp_p.tile([2 * CIN, PIECE], F32R, tag="xp")
                nc.sync.dma_start(xt[:, 0:pw], _ap(xp6_d, pc, [[1, pw]]))
                for nm in ("k", "v"):
                    pt = ps_pp.tile([128, 512], F32, tag="pp")
                    pst[nm, pi] = pt
                    nc.tensor.matmul(pt[:, 0:pw], w_t[nm], xt[:, 0:pw],
                                     start=True, stop=True)

            def piece_copy(pi):
                pc = pieces[pi]
                pw = min(PIECE, MAPC - pc)
                for nm, mp in (("v", v_map), ("k", k_map)):
                    pt = pst.pop((nm, pi))
                    if nm == "k" and pi < 2:
                        # off the v2->t1(0) chain; ACT is idle pre-e1(0)
                        nc.scalar.copy(mp[:, pc:pc + pw], pt[:, 0:pw])
                    else:
                        nc.vector.tensor_copy(mp[:, pc:pc + pw], pt[:, 0:pw])
                if pi % 2 == 0:
                    nc.vector.tensor_tensor(v2_map[:, pc:pc + pw],
                                            v_map[:, pc:pc + pw],
                                            v_map[:, pc:pc + pw], MULT)
                else:
                    nc.gpsimd.tensor_tensor(v2_map[:, pc:pc + pw],
                                            v_map[:, pc:pc + pw],
                                            v_map[:, pc:pc + pw], MULT)

            # Just-in-time piece schedule: piece pi's first consumer is
            # t1(j*) at cycle j*+1; emit it one cycle earlier.
            emit_at = {}
            for pi, pc in enumerate(pieces):
                r = pc // WP
                jstar = max(0, -(-(r - 5) // 2))
                emit_at.setdefault(max(0, jstar - 1), []).append(pi)

            # ==== phase B: attention chunks, software-pipelined ====
            # Stage offsets (chunk j): DMA/A/E @ cycle j; t1/e1 @ j+1;
            # r1/q-mm/rc1/qp/f @ j+2; s2/e2 @ j+4; m2/r2/r3/rc2/out @ j+5.
            # Per-engine emission order within a cycle keeps every queue head
            # on a >=1-cycle-old dependency (in-order engines never stall on
            # same-cycle work that sits behind them).
            st = {}             # (name, j) -> tile AP

            def win(mp, j, di):
                return _ap(mp[:], (j * ch + di) * WP, [[1, K], [WP, ch], [1, W]])

            def kkslice(t, di):
                return _ap(t[:], di * K * n, [[n, K], [W, ch], [1, W]])

            def reduce(src, acc, idt):
                for kk in range(KK):
                    nc.tensor.matmul(acc[:, 0:n], idt[:],
                                     src[:, kk * n:(kk + 1) * n],
                                     start=(kk == 0), stop=(kk == KK - 1))

            for cyc in range(nch + 6):
                jA = cyc            # DMA, A, E
                jB = cyc - 1        # t1, e1
                jC = cyc - 2        # r1, q-mm, rc1, qp, f
                jD = cyc - 4        # s2, e2
                jE = cyc - 5        # m2, r2, r3, rc2, out

                # --- SP/PE: JIT map-piece DMA + matmuls first ---
                for pi in emit_at.get(cyc, []):
                    piece_mm(pi)
                # --- Pool: m2(jE) (last chunk split with DVE to cut drain) ---
                if 0 <= jE < nch:
                    e2 = st["e2", jE]
                    m2 = kk_p.tile([128, KK * n], F16, tag="m2", bufs=2)
                    st["m2", jE] = m2
                    last = jE == nch - 1
                    for di in range(K):
                        eng = nc.vector if (last and di < 2) else nc.gpsimd
                        eng.tensor_tensor(
                            kkslice(m2, di), kkslice(e2, di),
                            win(v_map, jE, di), MULT)
                # --- DVE: s2(jD) first (f is 2 cycles old) ---
                if 0 <= jD < nch:
                    f_t = st.pop(("f", jD))
                    s2 = kk_p.tile([128, KK * n], F16, tag="s2", bufs=2)
                    st["s2", jD] = s2
                    for di in range(K):
                        nc.vector.tensor_tensor(
                            kkslice(s2, di), kkslice(f_t, di),
                            win(k_map, jD, di), MULT)
                # --- PE: r1(jC) first, then q-mm(jC), r2/r3(jE) ---
                if 0 <= jC < nch:
                    r1 = ps_r1.tile([128, 512], F32, tag="r1")
                    st["r1", jC] = r1
                    reduce(st["e1", jC], r1, id_bf)
                    q_ps = ps_mm.tile([128, 512], F32, tag="mm")
                    st["q", jC] = q_ps
                    nc.tensor.matmul(q_ps[:, 0:n], w_t["q"],
                                     st.pop(("xq", jC))[:],
                                     start=True, stop=True)
                # --- ACT: e2(jD) then e1(jB) ---
                if 0 <= jD < nch:
                    e2 = kk_p.tile([128, KK * n], F16, tag="e2", bufs=3)
                    st["e2", jD] = e2
                    s2 = st.pop(("s2", jD))
                    if jD == nch - 1:   # split so the tail m2 starts earlier
                        h = KK * n // 2
                        nc.scalar.activation(e2[:, 0:h], s2[:, 0:h], EXP)
                        nc.scalar.activation(e2[:, h:], s2[:, h:], EXP)
                    else:
                        nc.scalar.activation(e2[:], s2[:], EXP)
                # --- DVE: t1(jB) ---
                if 0 <= jB < nch:
                    E_t = st.pop(("E", jB))
                    t1 = kk_p.tile([128, KK * n], F16, tag="t1", bufs=2)
                    st["t1", jB] = t1
                    for di in range(K):
                        nc.vector.tensor_tensor(
                            kkslice(t1, di), win(v2_map, jB, di),
                            _ap(E_t[:], 0, [[0, K], [W, ch], [1, W]]), MULT)
                if 0 <= jB < nch:
                    e1 = kk_p.tile([128, KK * n], BF16, tag="e1", bufs=3)
                    st["e1", jB] = e1
                    t1 = st.pop(("t1", jB))
                    if jB == 0:     # split so the first exp starts earlier
                        h = KK * n // 2
                        nc.scalar.activation(e1[:, 0:h], t1[:, 0:h], EXP)
                        nc.scalar.activation(e1[:, h:], t1[:, h:], EXP)
                    else:
                        nc.scalar.activation(e1[:], t1[:], EXP)
                # --- PE: r2/r3(jE) ---
                if 0 <= jE < nch:
                    r2 = ps_r2.tile([128, 512], F32, tag="r2")
                    st["r2", jE] = r2
                    reduce(st.pop(("e2", jE)), r2, id_f16)
                    r3 = ps_r3.tile([128, 512], F32, tag="r3")
                    st["r3", jE] = r3
                    reduce(st.pop(("m2", jE)), r3, id_f16)
                # --- DVE: qp(jC) = q * 1/r1, then smalls(jA), then out(jE) ---
                if 0 <= jC < nch:
                    rc1 = sm_p.tile([128, n], F32, tag="rc1")
                    nc.vector.reciprocal_approx_fast(
                        rc1[:], st.pop(("r1", jC))[:, 0:n])
                    qp = sm_p.tile([128, n], BF16, tag="qp")
                    st["qp", jC] = qp
                    nc.vector.tensor_tensor(qp[:], st.pop(("q", jC))[:, 0:n],
                                            rc1[:], MULT)
                if 0 <= jA < nch:
                    E_t = sm_p.tile([128, n], F16, tag="E", bufs=3)
                    st["E", jA] = E_t
                    nc.sync.dma_start(E_t[:], _ap(em_d, jA * ch * W, [[1, n]]))
                    xq_t = sm_p.tile([2 * CIN, n], F32R, tag="xq", bufs=3)
                    st["xq", jA] = xq_t
                    nc.sync.dma_start(
                        xq_t[:], _ap(xp6_d, (jA * ch + PAD) * WP + PAD,
                                     [[WP, ch], [1, W]]))
                # --- Pool: f(jC) late (qp just produced by DVE) ---
                if 0 <= jC < nch:
                    f_t = kk_p.tile([128, KK * n], F16, tag="f", bufs=3)
                    st["f", jC] = f_t
                    nc.gpsimd.tensor_tensor(
                        _ap(f_t[:], 0, [[n, KK], [1, n]]),
                        _ap(st["e1", jC][:], 0, [[n, KK], [1, n]]),
                        _ap(st.pop(("qp", jC))[:], 0, [[0, KK], [1, n]]), MULT)
                    st.pop(("e1", jC))
                # --- DVE tail: out(jE) = r3 * 1/r2; SP: out DMA ---
                if 0 <= jE < nch:
                    rc2 = sm_p.tile([128, n], F32, tag="rc2")
                    nc.vector.reciprocal_approx_fast(
                        rc2[:], st.pop(("r2", jE))[:, 0:n])
                    out_t = sm_p.tile([128, n], F32, tag="out", bufs=3)
                    nc.vector.tensor_tensor(out_t[:],
                                            st.pop(("r3", jE))[:, 0:n],
                                            rc2[:], MULT)
                    for half in (0, 1):
                        nc.sync.dma_start(
                            _ap(out_d, (HH * half + jE * ch) * W, [[1, n]]),
                            out_t[C * half:C * (half + 1), :])
                # --- tail: map-piece copies (DVE/ACT) + v2 (DVE/Pool) ---
                for pi in emit_at.get(cyc, []):
                    piece_copy(pi)


_compiled_nc = None


def _get_nc():
    global _compiled_nc
    if _compiled_nc is None:
        nc = bacc.Bacc("TRN2", target_bir_lowering=False, debug=False,
                       num_devices=N_CORES)
        build_kernel(nc)
        nc.compile()
        _compiled_nc = nc
    return _compiled_nc


def _shard_inputs(x, q_w, k_w, v_w, emb_a, emb_b, emb_mix):
    xp = np.pad(x.astype(np.float32), ((0, 0), (0, 0), (PAD, PAD), (PAD, PAD)))
    # [B, 6, MAPC]: rows 0..2 = ci over padded rows 0..66 (half 0),
    #               rows 3..5 = ci over padded rows 64..130 (half 1)
    xp6 = np.concatenate([xp[:, :, 0:MAPR, :].reshape(B, CIN, MAPC),
                          xp[:, :, HH:HH + MAPR, :].reshape(B, CIN, MAPC)],
                         axis=1)

    def w6(wT):
        full = np.zeros((2 * CIN, 128), np.float32)
        full[0:CIN, 0:C] = wT
        full[CIN:2 * CIN, C:128] = wT
        return full

    # E = (ea + eb) * mix, fused host-side like the padding/packing prep.
    E = (emb_a[:, None, :] + emb_b[:, :, None]) * emb_mix      # [C, H, W]
    em2 = E.reshape(C, 2, HH, W).transpose(1, 0, 2, 3).reshape(128, HH * W)
    common = {
        "w6": np.ascontiguousarray(
            np.concatenate([w6(k_w.T), w6(v_w.T), w6(q_w.T)], axis=1)),
        "em2": np.ascontiguousarray(em2.astype(np.float16)),
    }
    return [dict(common, xp6=np.ascontiguousarray(xp6[b]))
            for b in range(B)]


def kernel(x, q_w, k_w, v_w, emb_a, emb_b, emb_mix):
    x, q_w, k_w, v_w, emb_a, emb_b, emb_mix = (
        np.asarray(a, dtype=np.float32)
        for a in (x, q_w, k_w, v_w, emb_a, emb_b, emb_mix))
    nc = _get_nc()
    in_maps = _shard_inputs(x, q_w, k_w, v_w, emb_a, emb_b, emb_mix)
    res = bass_utils.run_bass_kernel_spmd(nc, in_maps, list(range(N_CORES)))
    out = np.stack([res.results[b]["out"].reshape(C, H, W) for b in range(B)])
    return out.astype(np.float32)



# revision 66
# speedup vs baseline: 1.0188x; 1.0062x over previous
"""Trainium2 Bass kernel for nn_AttentionStem (sparse local attention stem).

Math per output element (b, c, h, w), window kk = (di, dj) in 4x4, PAD=2:
  E[c,h,w]   = (emb_a[c,w] + emb_b[c,h]) * emb_mix[c,h,w]
  e1_kk      = exp(v_kk^2 * E)                  (softmax-1 numerator)
  q'         = q / sum_kk(e1)                   (fold softmax-1 denom into q)
  e2_kk      = exp(q' * k_kk * e1_kk)           (softmax-2 numerator)
  out        = sum_kk(e2 * v_kk) / sum_kk(e2)

Sharding: pure data parallel, one batch element per NeuronCore (8 cores).
E is folded on the host (input prep, like the padding/weight packing).

Layout per core: SBUF partition p = 64*half + c (half = h<64 ? 0 : 1), free
dims stream (h, w).  k/v/v2 maps are built once, full-size ([128, 67*132]
fp16), from a 6-partition stacked padded input (both halves' rows), so each
1x1-conv piece is a single matmul; pieces are emitted just-in-time inside
the chunk pipeline so their PSUM->SBUF copies ride in engine slack.

The 32 row-chunks (ch=2 rows per half, n=512 positions) run through an
explicitly software-pipelined schedule — engines execute their instruction
streams in order, so stages of chunk j are emitted across cycles j..j+5
(t1/e1 @ j+1; r1/q/qp/f @ j+2; s2/e2 @ j+4; m2/r2/r3/out @ j+5) and ordered
within each cycle so every queue head waits only on >=1-cycle-old inputs.
Steady state is paced by the Activation engine running the two exps
back-to-back (~7.2us per cycle).  Engine balance (measured cost-model):
  ACT  (0.83 ns/el): e1 = exp(t1), e2 = exp(s2)            ~231 us
  DVE  (0.52 ns/el 2x fp16): t1/s2 windows, qp, out,
        reciprocals, most k/v PSUM->fp16 copies            ~223 us
  Pool (0.83 ns/el): f = e1*q' (kk-broadcast), m2 window   ~222 us
  PE   (0.42 ns/col): convs + 3 sum_kk chains of 16
        PSUM-accumulating identity matmuls (exact fp32)    ~177 us
"""
import sys, os
for _p in ("/opt/trn_rl_repo", "/root/.axon_site/_ro/trn_rl_repo"):
    if os.path.isdir(_p) and _p not in sys.path:
        sys.path.insert(0, _p)

from contextlib import ExitStack, nullcontext as _nullcm
import numpy as np

import concourse.bass as bass
import concourse.bacc as bacc
import concourse.tile as tile
from concourse import mybir
import concourse.bass_utils as bass_utils
from concourse.bass_types import AP
from concourse import masks

N_CORES = 8
B, CIN, H, W = 8, 3, 128, 128
C = 64
K, PAD, KK = 4, 2, 16
HH = H // 2                 # rows per half (64)
WP = W + 2 * PAD            # 132
MAPR = HH + K - 1           # map rows kept per half (67)
MAPC = MAPR * WP            # map cols per partition (8844)
CH = 2                      # h-rows per half per chunk
PIECE = 492                 # conv piece (<=512 psum cols), ceil(8844/18)

F32 = mybir.dt.float32
BF16 = mybir.dt.bfloat16
F16 = mybir.dt.float16
F32R = mybir.dt.float32r
MULT = mybir.AluOpType.mult
EXP = mybir.ActivationFunctionType.Exp


def _ap(base: AP, offset: int, dims):
    """Build a custom free-dim AP on a tile/dram AP, keeping its partition dim."""
    return AP(tensor=base.tensor, offset=base.offset + offset,
              ap=[list(base.ap[0])] + [list(d) for d in dims])


def build_kernel(nc, ch: int = CH, cfg=None, reps: int = 0):
    """reps>0 wraps the whole body in a hardware loop (for benchmarking)."""
    n = ch * W                      # spatial elems per partition per chunk
    nch = HH // ch                  # chunks

    xp6_d = nc.dram_tensor("xp6", [2 * CIN, MAPC], F32R, kind="ExternalInput").ap()
    w6_d = nc.dram_tensor("w6", [2 * CIN, 3 * 128], F32R,
                          kind="ExternalInput").ap()
    em_d = nc.dram_tensor("em2", [128, HH * W], F16, kind="ExternalInput").ap()
    out_d = nc.dram_tensor("out", [C, H * W], F32, kind="ExternalOutput").ap()

    with tile.TileContext(nc) as tc, ExitStack() as ctx:
        const = ctx.enter_context(tc.tile_pool(name="const", bufs=1))
        xp_p = ctx.enter_context(tc.tile_pool(name="xp", bufs=3))
        map_p = ctx.enter_context(tc.tile_pool(name="maps", bufs=1))
        kk_p = ctx.enter_context(tc.tile_pool(name="kk", bufs=6))
        sm_p = ctx.enter_context(tc.tile_pool(name="small", bufs=2))
        ps_mm = ctx.enter_context(tc.tile_pool(name="psmm", bufs=2, space="PSUM"))
        ps_pp = ctx.enter_context(tc.tile_pool(name="pspp", bufs=3, space="PSUM"))
        ps_r1 = ctx.enter_context(tc.tile_pool(name="psr1", bufs=1, space="PSUM"))
        ps_r2 = ctx.enter_context(tc.tile_pool(name="psr2", bufs=1, space="PSUM"))
        ps_r3 = ctx.enter_context(tc.tile_pool(name="psr3", bufs=1, space="PSUM"))

        # ---- constants (k/v/q weights in one DMA) ----
        w_all = const.tile([2 * CIN, 3 * 128], F32R, tag="w6")
        nc.sync.dma_start(w_all[:], w6_d[:])
        w_t = {nm: w_all[:, 128 * i:128 * (i + 1)]
               for i, nm in enumerate(("k", "v", "q"))}
        ident = const.tile([128, 128], F32, tag="ident")
        masks.make_identity(nc, ident[:])
        id_bf = const.tile([128, 128], BF16, tag="idbf")
        nc.vector.tensor_copy(id_bf[:], ident[:])
        id_f16 = const.tile([128, 128], F16, tag="idf16")
        nc.vector.tensor_copy(id_f16[:], ident[:])

        loop_cm = tc.For_i(0, reps, 1) if reps else _nullcm()
        with loop_cm:
            # ==== k/v/v2 full maps; pieces emitted inside the early cycles ====
            k_map = map_p.tile([128, MAPC], F16, tag="kmap")
            v_map = map_p.tile([128, MAPC], F16, tag="vmap")
            v2_map = map_p.tile([128, MAPC], F16, tag="v2map")

            pieces = list(range(0, MAPC, PIECE))
            pst = {}

            def piece_mm(pi):
                pc = pieces[pi]
                pw = min(PIECE, MAPC - pc)
                xt = xp_p.tile([2 * CIN, PIECE], F32R, tag="xp")
                nc.sync.dma_start(xt[:, 0:pw], _ap(xp6_d, pc, [[1, pw]]))
                for nm in ("v", "k"):
                    pt = ps_pp.tile([128, 512], F32, tag="pp")
                    pst[nm, pi] = pt
                    nc.tensor.matmul(pt[:, 0:pw], w_t[nm], xt[:, 0:pw],
                                     start=True, stop=True)

            def piece_copy(pi):
                pc = pieces[pi]
                pw = min(PIECE, MAPC - pc)
                for nm, mp in (("v", v_map), ("k", k_map)):
                    pt = pst.pop((nm, pi))
                    if nm == "k" and pi < 2:
                        # off the v2->t1(0) chain; ACT is idle pre-e1(0)
                        nc.scalar.copy(mp[:, pc:pc + pw], pt[:, 0:pw])
                    else:
                        nc.vector.tensor_copy(mp[:, pc:pc + pw], pt[:, 0:pw])
                if pi % 2 == 0:
                    nc.vector.tensor_tensor(v2_map[:, pc:pc + pw],
                                            v_map[:, pc:pc + pw],
                                            v_map[:, pc:pc + pw], MULT)
                else:
                    nc.gpsimd.tensor_tensor(v2_map[:, pc:pc + pw],
                                            v_map[:, pc:pc + pw],
                                            v_map[:, pc:pc + pw], MULT)

            # Just-in-time piece schedule: piece pi's first consumer is
            # t1(j*) at cycle j*+1; emit it one cycle earlier.
            emit_at = {}
            for pi, pc in enumerate(pieces):
                r = pc // WP
                jstar = max(0, -(-(r - 5) // 2))
                emit_at.setdefault(max(0, jstar - 1), []).append(pi)

            # ==== phase B: attention chunks, software-pipelined ====
            # Stage offsets (chunk j): DMA/A/E @ cycle j; t1/e1 @ j+1;
            # r1/q-mm/rc1/qp/f @ j+2; s2/e2 @ j+4; m2/r2/r3/rc2/out @ j+5.
            # Per-engine emission order within a cycle keeps every queue head
            # on a >=1-cycle-old dependency (in-order engines never stall on
            # same-cycle work that sits behind them).
            st = {}             # (name, j) -> tile AP

            def win(mp, j, di):
                return _ap(mp[:], (j * ch + di) * WP, [[1, K], [WP, ch], [1, W]])

            def kkslice(t, di):
                return _ap(t[:], di * K * n, [[n, K], [W, ch], [1, W]])

            def reduce(src, acc, idt):
                for kk in range(KK):
                    nc.tensor.matmul(acc[:, 0:n], idt[:],
                                     src[:, kk * n:(kk + 1) * n],
                                     start=(kk == 0), stop=(kk == KK - 1))

            for cyc in range(nch + 6):
                jA = cyc            # DMA, A, E
                jB = cyc - 1        # t1, e1
                jC = cyc - 2        # r1, q-mm, rc1, qp, f
                jD = cyc - 4        # s2, e2
                jE = cyc - 5        # m2, r2, r3, rc2, out

                # --- SP/PE: JIT map-piece DMA + matmuls first ---
                for pi in emit_at.get(cyc, []):
                    piece_mm(pi)
                # --- Pool: m2(jE) (last chunk split with DVE to cut drain) ---
                if 0 <= jE < nch:
                    e2 = st["e2", jE]
                    m2 = kk_p.tile([128, KK * n], F16, tag="m2", bufs=2)
                    st["m2", jE] = m2
                    last = jE == nch - 1
                    for di in range(K):
                        eng = nc.vector if (last and di < 2) else nc.gpsimd
                        eng.tensor_tensor(
                            kkslice(m2, di), kkslice(e2, di),
                            win(v_map, jE, di), MULT)
                # --- DVE: s2(jD) first (f is 2 cycles old) ---
                if 0 <= jD < nch:
                    f_t = st.pop(("f", jD))
                    s2 = kk_p.tile([128, KK * n], F16, tag="s2", bufs=2)
                    st["s2", jD] = s2
                    for di in range(K):
                        nc.vector.tensor_tensor(
                            kkslice(s2, di), kkslice(f_t, di),
                            win(k_map, jD, di), MULT)
                # --- PE: r1(jC) first, then q-mm(jC), r2/r3(jE) ---
                if 0 <= jC < nch:
                    r1 = ps_r1.tile([128, 512], F32, tag="r1")
                    st["r1", jC] = r1
                    reduce(st["e1", jC], r1, id_bf)
                    q_ps = ps_mm.tile([128, 512], F32, tag="mm")
                    st["q", jC] = q_ps
                    nc.tensor.matmul(q_ps[:, 0:n], w_t["q"],
                                     st.pop(("xq", jC))[:],
                                     start=True, stop=True)
                # --- ACT: e2(jD) then e1(jB) ---
                if 0 <= jD < nch:
                    e2 = kk_p.tile([128, KK * n], F16, tag="e2", bufs=3)
                    st["e2", jD] = e2
                    s2 = st.pop(("s2", jD))
                    if jD == nch - 1:   # split so the tail m2 starts earlier
                        h = KK * n // 2
                        nc.scalar.activation(e2[:, 0:h], s2[:, 0:h], EXP)
                        nc.scalar.activation(e2[:, h:], s2[:, h:], EXP)
                    else:
                        nc.scalar.activation(e2[:], s2[:], EXP)
                # --- DVE: t1(jB) ---
                if 0 <= jB < nch:
                    E_t = st.pop(("E", jB))
                    t1 = kk_p.tile([128, KK * n], F16, tag="t1", bufs=2)
                    st["t1", jB] = t1
                    for di in range(K):
                        nc.vector.tensor_tensor(
                            kkslice(t1, di), win(v2_map, jB, di),
                            _ap(E_t[:], 0, [[0, K], [W, ch], [1, W]]), MULT)
                if 0 <= jB < nch:
                    e1 = kk_p.tile([128, KK * n], BF16, tag="e1", bufs=3)
                    st["e1", jB] = e1
                    t1 = st.pop(("t1", jB))
                    if jB == 0:     # split so the first exp starts earlier
                        h = KK * n // 2
                        nc.scalar.activation(e1[:, 0:h], t1[:, 0:h], EXP)
                        nc.scalar.activation(e1[:, h:], t1[:, h:], EXP)
                    else:
                        nc.scalar.activation(e1[:], t1[:], EXP)
                # --- PE: r2/r3(jE) ---
                if 0 <= jE < nch:
                    r2 = ps_r2.tile([128, 512], F32, tag="r2")
                    st["r2", jE] = r2
                    reduce(st.pop(("e2", jE)), r2, id_f16)
                    r3 = ps_r3.tile([128, 512], F32, tag="r3")
                    st["r3", jE] = r3
                    reduce(st.pop(("m2", jE)), r3, id_f16)
                # --- DVE: qp(jC) = q * 1/r1, then smalls(jA), then out(jE) ---
                if 0 <= jC < nch:
                    rc1 = sm_p.tile([128, n], F32, tag="rc1")
                    nc.vector.reciprocal_approx_fast(
                        rc1[:], st.pop(("r1", jC))[:, 0:n])
                    qp = sm_p.tile([128, n], BF16, tag="qp")
                    st["qp", jC] = qp
                    nc.vector.tensor_tensor(qp[:], st.pop(("q", jC))[:, 0:n],
                                            rc1[:], MULT)
                if 0 <= jA < nch:
                    E_t = sm_p.tile([128, n], F16, tag="E", bufs=3)
                    st["E", jA] = E_t
                    nc.sync.dma_start(E_t[:], _ap(em_d, jA * ch * W, [[1, n]]))
                    xq_t = sm_p.tile([2 * CIN, n], F32R, tag="xq", bufs=3)
                    st["xq", jA] = xq_t
                    nc.sync.dma_start(
                        xq_t[:], _ap(xp6_d, (jA * ch + PAD) * WP + PAD,
                                     [[WP, ch], [1, W]]))
                # --- Pool: f(jC) late (qp just produced by DVE) ---
                if 0 <= jC < nch:
                    f_t = kk_p.tile([128, KK * n], F16, tag="f", bufs=3)
                    st["f", jC] = f_t
                    nc.gpsimd.tensor_tensor(
                        _ap(f_t[:], 0, [[n, KK], [1, n]]),
                        _ap(st["e1", jC][:], 0, [[n, KK], [1, n]]),
                        _ap(st.pop(("qp", jC))[:], 0, [[0, KK], [1, n]]), MULT)
                    st.pop(("e1", jC))
                # --- DVE tail: out(jE) = r3 * 1/r2; SP: out DMA ---
                if 0 <= jE < nch:
                    rc2 = sm_p.tile([128, n], F32, tag="rc2")
                    nc.vector.reciprocal_approx_fast(
                        rc2[:], st.pop(("r2", jE))[:, 0:n])
                    out_t = sm_p.tile([128, n], F32, tag="out", bufs=3)
                    nc.vector.tensor_tensor(out_t[:],
                                            st.pop(("r3", jE))[:, 0:n],
                                            rc2[:], MULT)
                    for half in (0, 1):
                        nc.sync.dma_start(
                            _ap(out_d, (HH * half + jE * ch) * W, [[1, n]]),
                            out_t[C * half:C * (half + 1), :])
                # --- tail: map-piece copies (DVE/ACT) + v2 (DVE/Pool) ---
                for pi in emit_at.get(cyc, []):
                    piece_copy(pi)


_compiled_nc = None


def _get_nc():
    global _compiled_nc
    if _compiled_nc is None:
        nc = bacc.Bacc("TRN2", target_bir_lowering=False, debug=False,
                       num_devices=N_CORES)
        build_kernel(nc)
        nc.compile()
        _compiled_nc = nc
    return _compiled_nc


def _shard_inputs(x, q_w, k_w, v_w, emb_a, emb_b, emb_mix):
    xp = np.pad(x.astype(np.float32), ((0, 0), (0, 0), (PAD, PAD), (PAD, PAD)))
    # [B, 6, MAPC]: rows 0..2 = ci over padded rows 0..66 (half 0),
    #               rows 3..5 = ci over padded rows 64..130 (half 1)
    xp6 = np.concatenate([xp[:, :, 0:MAPR, :].reshape(B, CIN, MAPC),
                          xp[:, :, HH:HH + MAPR, :].reshape(B, CIN, MAPC)],
                         axis=1)

    def w6(wT):
        full = np.zeros((2 * CIN, 128), np.float32)
        full[0:CIN, 0:C] = wT
        full[CIN:2 * CIN, C:128] = wT
        return full

    # E = (ea + eb) * mix, fused host-side like the padding/packing prep.
    E = (emb_a[:, None, :] + emb_b[:, :, None]) * emb_mix      # [C, H, W]
    em2 = E.reshape(C, 2, HH, W).transpose(1, 0, 2, 3).reshape(128, HH * W)
    common = {
        "w6": np.ascontiguousarray(
            np.concatenate([w6(k_w.T), w6(v_w.T), w6(q_w.T)], axis=1)),
        "em2": np.ascontiguousarray(em2.astype(np.float16)),
    }
    return [dict(common, xp6=np.ascontiguousarray(xp6[b]))
            for b in range(B)]


def kernel(x, q_w, k_w, v_w, emb_a, emb_b, emb_mix):
    x, q_w, k_w, v_w, emb_a, emb_b, emb_mix = (
        np.asarray(a, dtype=np.float32)
        for a in (x, q_w, k_w, v_w, emb_a, emb_b, emb_mix))
    nc = _get_nc()
    in_maps = _shard_inputs(x, q_w, k_w, v_w, emb_a, emb_b, emb_mix)
    res = bass_utils.run_bass_kernel_spmd(nc, in_maps, list(range(N_CORES)))
    out = np.stack([res.results[b]["out"].reshape(C, H, W) for b in range(B)])
    return out.astype(np.float32)

